# revision 15
# baseline (speedup 1.0000x reference)
"""Trainium2 Bass kernel for binarized BERT self-attention (BiT-style).

Reference math:
  q = sign(h)*a_q @ (sign(Wq)*mean|Wq|).T + bq     (binarized linear)
  q2 = sign(q)*clip_q   (same for k, v)
  p  = softmax(q2 k2^T / sqrt(D) + mask)
  pq = clip(round(p/clip_a), 0, 1) * clip_a        (binary attention probs)
  out = pq @ v2

Exact algebraic facts used:
  * sign values pack as +-0.5 (exact in fp8e4m3/bf16); every matmul here is
    exact small-integer arithmetic accumulated in fp32 PSUM.
  * sign(x@W.T + b) thresholds the packed-sign matmul: (M/4) > -b/(4*a*s).
  * pq nonzero iff exp(s_i) > 0.5*clip_a * sum_j exp(s_j); scores bounded so
    no softmax max-pass is needed.  round() half-to-even matches strict '>'.

Sharding (8 cores): core = (batch b, head-group g); each core computes 8
heads x 1024 tokens and returns ctx^T [512 head-cols, 1024 tokens].

Performance structure (CoreSim cost-model driven, software-pipelined):
  * Scores run fp8 DoubleRow (0.5 cyc/row): the D=64 contraction is folded
    as [Ki=32 partitions, Ko=2]; Wq/Wk columns are host-permuted so each
    projection m-tile lands the fold layout directly.
  * exp() split across Scalar (chunks 0-6) and Pool (chunk 7) engines.
  * Softmax-sum matmul runs over pair-folded E (DVE folds) -> half PE work.
  * Prob compares: chunks 0-3 on DVE (bf16 P), chunks 4-7 on Pool writing
    fp8 P, so half the context matmul runs fp8 DoubleRow.
  * Emission is software-pipelined: head h's exp stream overlaps head h-1's
    compare/context tail, V-projection and late QKV tiles fill the first
    two head windows, and all DMAs are batched with the critical h/wq/wk
    bytes front-loaded (bf16 delivery, sign-preserving).
"""

import math

import numpy as np

B, S, H, NH, D = 4, 1024, 1024, 16, 64
NCORES, G = 8, 2
HG = H // G  # 512 output columns per core (8 heads)
EPS = 1e-5
KC = H // 128  # 8 contraction chunks
TC = S // 128  # 8 token chunks
MC = HG // 128  # 4 output chunks per core


def _qk_perm():
    """Column order for Wq/Wk shards: m-tile m=(2*hq+o), partition p=32*b+r
    holds neuron (4*hq+b)*64 + o*32 + r, so the projection psum directly
    yields the [Ki=32, Ko=2] DoubleRow fold for the scores contraction."""
    perm = np.empty(HG, dtype=np.int64)
    for m in range(MC):
        hq, o = m // 2, m % 2
        for p in range(128):
            b, r = p // 32, p % 32
            perm[m * 128 + p] = (4 * hq + b) * 64 + o * 32 + r
    return perm


_PERM = _qk_perm()


def _split_multi_waits(nc):
    """Walrus accepts at most ONE sync-wait per instruction.  Move all but
    one wait onto preceding same-engine NOPs (semantically equivalent since
    engines execute their streams in order)."""
    from concourse import mybir

    eng_api = {
        mybir.EngineType.PE: nc.tensor,
        mybir.EngineType.DVE: nc.vector,
        mybir.EngineType.Activation: nc.scalar,
        mybir.EngineType.Pool: nc.gpsimd,
        mybir.EngineType.SP: nc.sync,
    }

    plan = []
    for f in nc.m.functions:
        for bb in f.blocks:
            for ins in bb.instructions:
                si = ins.sync_info
                if si is None or not si.on_wait or len(si.on_wait) <= 1:
                    continue
                plan.append((f, bb, ins))

    fillers = {}
    for f, bb, ins in plan:
        si = ins.sync_info
        waits = list(si.on_wait)
        nops = []
        for w in waits[:-1]:
            bi = eng_api[ins.engine].nop()
            raw = bi.ins
            raw.sync_info = mybir.SyncInfo(on_wait=[w], on_update=[])
            nops.append(raw)
        ins.sync_info = mybir.SyncInfo(
            on_wait=[waits[-1]], on_update=list(si.on_update or [])
        )
        fillers[ins.name] = nops

    created = {n.name for nops in fillers.values() for n in nops}
    for f in nc.m.functions:
        for bb in f.blocks:
            out = []
            for ins in bb.instructions:
                if ins.name in created:
                    continue
                out.extend(fillers.get(ins.name, ()))
                out.append(ins)
            bb.instructions = out
    return nc


def _build_program(exp_scale: float, th_scale: float, out_scale: float,
                   masked: bool):
    import concourse.bass as bass
    import concourse.tile as tile
    from concourse import mybir

    f32, bf16 = mybir.dt.float32, mybir.dt.bfloat16
    fp8 = mybir.dt.float8e4
    DR = mybir.MatmulPerfMode.DoubleRow
    gt = mybir.AluOpType.is_gt
    sub = mybir.AluOpType.subtract
    mult = mybir.AluOpType.mult
    add = mybir.AluOpType.add
    Exp = mybir.ActivationFunctionType.Exp

    nc = bass.Bass()
    hT_d = nc.dram_tensor("hT", [H, S], bf16, kind="ExternalInput")
    wT_d = {
        w: nc.dram_tensor(f"w{w}T", [H, HG], bf16, kind="ExternalInput")
        for w in "qkv"
    }
    thrq_d = nc.dram_tensor("thrq", [HG], f32, kind="ExternalInput")
    thrk_d = nc.dram_tensor("thrk", [HG], f32, kind="ExternalInput")
    bvrow_d = nc.dram_tensor("bvrow", [HG], f32, kind="ExternalInput")
    mask_d = nc.dram_tensor("mask", [S], f32, kind="ExternalInput")
    out_d = nc.dram_tensor("ctxT", [HG, S], f32, kind="ExternalOutput")

    with tile.TileContext(nc) as tc:
        with (
            tc.tile_pool(name="persist", bufs=1) as persist,
            tc.tile_pool(name="heads", bufs=2) as headp,
            tc.tile_pool(name="ps_s", bufs=2, space="PSUM") as ps_s,
            tc.tile_pool(name="ps_t", bufs=1, space="PSUM") as ps_t,
            tc.tile_pool(name="ps_c", bufs=1, space="PSUM") as ps_c,
        ):
            hstage = persist.tile([128, KC, S], bf16, tag="hstage")
            wstage = {
                w: persist.tile(
                    [128, KC, HG], bf16, tag=f"wstage_{w}", name=f"wstage_{w}"
                )
                for w in "qkv"
            }
            shT = persist.tile([128, KC, S], fp8, tag="shT")
            swT = {
                w: persist.tile(
                    [128, KC, HG], fp8, tag=f"swT_{w}", name=f"swT_{w}"
                )
                for w in "qkv"
            }
            # fold layout: [32*b + r, hq, o, token]; head 4*hq+b, d = o*32+r
            qT4 = persist.tile([128, 2, 2, S], fp8, tag="qT4")
            kT4 = persist.tile([128, 2, 2, S], fp8, tag="kT4")
            v_bf = persist.tile([128, 4, HG], bf16, tag="v_bf")  # t=0..3
            v_q8 = persist.tile([128, 4, HG], fp8, tag="v_q8")  # t=4..7
            thrq_sb = persist.tile([128, MC], f32, tag="thrq")
            thrk_sb = persist.tile([128, MC], f32, tag="thrk")
            bvrow_sb = persist.tile([1, HG], f32, tag="bvrow")
            bvrow16 = persist.tile([1, HG], bf16, tag="bvrow16")
            mask_sb = persist.tile([128, TC], f32, tag="mask")
            ones16 = persist.tile([1, 128], bf16, tag="ones16")
            onesK = persist.tile([128, 128], bf16, tag="onesK")
            out_sb = persist.tile([128, MC, S], f32, tag="out_sb")

            # score/projection psum comes from a 2-buffer pool (2 banks per
            # buffer -> 4 banks total); Tps/Cps are persistent single tiles.
            Tps = ps_t.tile([128, S], f32, tag="Tps")  # 2 banks
            Cps = ps_c.tile([128, S], f32, tag="Cps")  # 2 banks

            # warmup: memsets on DVE, then dummy matmuls keep the PE
            # continuously busy through the load phase so the p-state ramp
            # completes before the first real projection; a dummy exp
            # pre-loads the activation table off the critical path.
            nc.vector.memset(onesK, 1.0)
            nc.vector.memset(ones16, 1.0)
            warm_sb = persist.tile([1, 128], bf16, tag="warm_sb")
            nc.scalar.activation(warm_sb, ones16, Exp, bias=0.0, scale=0.0)
            for _ in range(120):
                nc.tensor.matmul(
                    Tps[:, 0:128], lhsT=onesK, rhs=onesK,
                    start=True, stop=True,
                )
            # tiny control tensors first so their (negligible) transfers
            # clear the DMA device before the bulk loads queue up.
            nc.gpsimd.dma_start(
                out=thrq_sb, in_=thrq_d.rearrange("(m p) -> p m", p=128)
            )
            nc.gpsimd.dma_start(
                out=thrk_sb, in_=thrk_d.rearrange("(m p) -> p m", p=128)
            )
            nc.gpsimd.dma_start(
                out=bvrow_sb, in_=bvrow_d.rearrange("(o n) -> o n", o=1)
            )
            nc.gpsimd.memset(mask_sb, 0.0)
            if masked:
                nc.gpsimd.dma_start(
                    out=mask_sb, in_=mask_d.rearrange("(t p) -> p t", p=128)
                )
            nc.vector.tensor_copy(bvrow16, bvrow_sb)

            # --- input DMAs.  All bulk loads issue on SP in strict
            # priority order (h quarters, then the m0/m1 halves of wq/wk,
            # then the rest) -- a single issuer guarantees DMA-device order
            # so nothing steals slots from the critical path.
            AHG = HG // 2  # 256: columns of m-tiles 0,1
            for qtr in range(4):
                nc.sync.dma_start(
                    out=hstage[:, 2 * qtr : 2 * qtr + 2, :],
                    in_=hT_d[qtr * 256 : (qtr + 1) * 256, :].rearrange(
                        "(c p) s -> p c s", p=128
                    ),
                )
                eng = nc.vector if qtr % 2 == 0 else nc.gpsimd
                eng.tensor_scalar(
                    shT[:, 2 * qtr : 2 * qtr + 2, :],
                    hstage[:, 2 * qtr : 2 * qtr + 2, :],
                    0.0,
                    0.5,
                    gt,
                    sub,
                )
            for w in ("k", "q"):
                nc.sync.dma_start(
                    out=wstage[w][:, :, 0:AHG],
                    in_=wT_d[w][:, 0:AHG].rearrange("(c p) o -> p c o", p=128),
                )
                nc.vector.tensor_scalar(
                    swT[w][:, :, 0:AHG], wstage[w][:, :, 0:AHG],
                    0.0, 0.5, gt, sub,
                )
            for w in ("k", "q"):
                nc.sync.dma_start(
                    out=wstage[w][:, :, AHG:HG],
                    in_=wT_d[w][:, AHG:HG].rearrange(
                        "(c p) o -> p c o", p=128
                    ),
                )
            nc.sync.dma_start(
                out=wstage["v"],
                in_=wT_d["v"].rearrange("(c p) o -> p c o", p=128),
            )
            nc.vector.tensor_scalar(
                swT["v"], wstage["v"], 0.0, 0.5, gt, sub
            )

            # --- emission helpers -------------------------------------
            # score/projection psum tiles cycle through the 2-buffer pool;
            # WAR hazards between consecutive users are tracked per tile.
            slot_counter = [0]

            def next_slot():
                slot_counter[0] += 1
                return ps_s.tile(
                    [128, S], f32, tag="S", name=f"S{slot_counter[0]}"
                )

            def qk_mtile(w, m, pack_eng, ps=None):
                """Q/K projection m-tile into a psum slot, pack to fp8."""
                dstT = qT4 if w == "q" else kT4
                thr = thrq_sb if w == "q" else thrk_sb
                if ps is None:
                    ps = next_slot()
                for half in range(2):
                    sl = slice(half * 512, (half + 1) * 512)
                    for c2 in range(KC // 2):
                        nc.tensor.matmul(
                            ps[:, sl],
                            lhsT=swT[w][
                                :, 2 * c2 : 2 * c2 + 2, m * 128 : (m + 1) * 128
                            ],
                            rhs=shT[:, 2 * c2 : 2 * c2 + 2, sl],
                            start=(c2 == 0),
                            stop=(c2 == KC // 2 - 1),
                            perf_mode=DR,
                        )
                pack_eng.tensor_scalar(
                    dstT[:, m // 2, m % 2, :], ps, thr[:, m : m + 1], 0.5,
                    gt, sub,
                )

            def v_ttile(t, pack_eng=None):
                """V projection token-tile via a Cps half-slot; bf16 for
                t<4 (bf16 ctx chunks), fp8 for t>=4 (DoubleRow ctx pairs)."""
                ps = Cps[:, (t % 2) * 512 : (t % 2 + 1) * 512]
                for c2 in range(KC // 2):
                    nc.tensor.matmul(
                        ps,
                        lhsT=shT[
                            :, 2 * c2 : 2 * c2 + 2, t * 128 : (t + 1) * 128
                        ],
                        rhs=swT["v"][:, 2 * c2 : 2 * c2 + 2, :],
                        start=(c2 == 0),
                        stop=False,
                        perf_mode=DR,
                    )
                nc.tensor.matmul(
                    ps, lhsT=ones16, rhs=bvrow16, start=False, stop=True
                )
                dst = v_bf[:, t, :] if t < 4 else v_q8[:, t - 4, :]
                (pack_eng or nc.gpsimd).tensor_scalar(
                    dst, ps, 0.0, 0.5, gt, sub
                )

            # head-local state for the pipelined tail
            hstate = {}

            def sc_exp(h, c):
                st = hstate[h]
                # chunk 2 borrows the Tps banks (free between th(h-1) and
                # this window's first sum) -- a third slot that breaks the
                # 2-slot WAR chain pacing the activation stream.
                ps = Tps if c == 2 else next_slot()
                kv, qv = st["kv"], st["qv"]
                for sp in range(2):
                    sl = slice(sp * 512, (sp + 1) * 512)
                    nc.tensor.matmul(
                        ps[:, sl],
                        lhsT=kv[:, :, c * 128 : (c + 1) * 128],
                        rhs=qv[:, :, sl],
                        start=True,
                        stop=True,
                        perf_mode=DR,
                        tile_position=(32 * st["b"], 0),
                    )
                if c < 6:
                    nc.scalar.activation(
                        st["E"][:, c, :], ps, Exp,
                        bias=mask_sb[:, c : c + 1], scale=exp_scale,
                    )
                else:
                    eeng = nc.gpsimd
                    eeng.add_instruction(
                        mybir.InstActivation(
                            name=nc.get_next_instruction_name(),
                            func=Exp,
                            ins=[
                                eeng.lower_ap(ps),
                                eeng.lower_ap(mask_sb[:, c : c + 1]),
                                mybir.ImmediateValue(
                                    dtype=f32, value=exp_scale
                                ),
                                mybir.ImmediateValue(dtype=f32, value=0.0),
                            ],
                            outs=[eeng.lower_ap(st["E"][:, c, :])],
                        )
                    )

            def fold_sum(h, p, direct=False):
                st = hstate[h]
                if direct:
                    # skip the fold: sum the two raw chunks (shorter latency
                    # for the final head's tail at slightly more PE work)
                    for ci in range(2):
                        for sp in range(2):
                            sl = slice(sp * 512, (sp + 1) * 512)
                            nc.tensor.matmul(
                                Tps[:, sl],
                                lhsT=onesK,
                                rhs=st["E"][:, 2 * p + ci, sl],
                                start=(p == 0 and ci == 0),
                                stop=(p == TC // 2 - 1 and ci == 1),
                            )
                    return
                nc.vector.tensor_tensor(
                    st["Efold"][:, p, :],
                    st["E"][:, 2 * p, :],
                    st["E"][:, 2 * p + 1, :],
                    add,
                )
                for sp in range(2):
                    sl = slice(sp * 512, (sp + 1) * 512)
                    nc.tensor.matmul(
                        Tps[:, sl],
                        lhsT=onesK,
                        rhs=st["Efold"][:, p, sl],
                        start=(p == 0),
                        stop=(p == TC // 2 - 1),
                    )

            def th(h):
                st = hstate[h]
                # halves on DVE and Pool in parallel to halve the latency
                nc.vector.tensor_scalar(
                    st["Th"][:, 0:512], Tps[:, 0:512], th_scale, None, mult
                )
                nc.gpsimd.tensor_scalar(
                    st["Th"][:, 512:1024], Tps[:, 512:1024],
                    th_scale, None, mult,
                )

            def cmp_ctx(h, c, start_c, stop_c):
                st = hstate[h]
                hl, hp = st["hl"], st["hp"]
                if c < 4:
                    nc.vector.tensor_tensor(
                        st["P"][:, c, :], st["E"][:, c, :], st["Th"], gt
                    )
                    for sp in range(2):
                        sl = slice(sp * 512, (sp + 1) * 512)
                        nc.tensor.matmul(
                            Cps[hp : hp + 64, sl],
                            lhsT=v_bf[:, c, hl * 64 : (hl + 1) * 64],
                            rhs=st["P"][:, c, sl],
                            start=(c == start_c),
                            stop=(c == stop_c),
                            tile_position=(0, hp),
                        )
                else:
                    nc.gpsimd.tensor_tensor(
                        st["Pq"][:, c - 4, :], st["E"][:, c, :], st["Th"], gt
                    )
                    if c in (5, 7):  # DoubleRow pair (c-1, c)
                        pr = (c - 5) // 2  # 0 or 1
                        for sp in range(2):
                            sl = slice(sp * 512, (sp + 1) * 512)
                            nc.tensor.matmul(
                                Cps[hp : hp + 64, sl],
                                lhsT=v_q8[
                                    :, 2 * pr : 2 * pr + 2,
                                    hl * 64 : (hl + 1) * 64,
                                ],
                                rhs=st["Pq"][:, 2 * pr : 2 * pr + 2, sl],
                                start=(c == start_c),
                                stop=(c == stop_c),
                                perf_mode=DR,
                                tile_position=(0, hp),
                            )

            def out_m(m):
                for half in range(2):
                    sl = slice(half * 512, (half + 1) * 512)
                    eng = nc.gpsimd if half == 0 else nc.vector
                    eng.tensor_scalar(
                        out_sb[:, m, sl], Cps[:, sl], out_scale, None, mult
                    )
                    nc.sync.dma_start(
                        out=out_d.rearrange("(m p) s -> p m s", p=128)[
                            :, m, sl
                        ],
                        in_=out_sb[:, m, sl],
                    )

            def open_head(h):
                hl = h
                b, hq = hl % 4, hl // 4
                hstate[h] = {
                    "hl": hl,
                    "b": b,
                    "hp": 64 * (hl % 2),
                    "kv": kT4[32 * b : 32 * b + 32, hq, :, :],
                    "qv": qT4[32 * b : 32 * b + 32, hq, :, :],
                    "E": headp.tile([128, TC, S], bf16, tag="E", name=f"E_{h}"),
                    "Efold": headp.tile(
                        [128, TC // 2, S], bf16, tag="Ef", name=f"Ef_{h}"
                    ),
                    "P": headp.tile([128, 4, S], bf16, tag="P", name=f"P_{h}"),
                    "Pq": headp.tile(
                        [128, 4, S], fp8, tag="Pq", name=f"Pq_{h}"
                    ),
                    "Th": headp.tile([128, S], bf16, tag="Th", name=f"Th_{h}"),
                }

            # --- prelude QKV: m-tiles 0,1 of k then q (enough for heads
            # 0-3); k1 borrows the Tps banks so all four tiles pipeline
            # without slot-reuse stalls; packs split across DVE and Pool.
            qk_mtile("k", 0, nc.vector)
            qk_mtile("k", 1, nc.gpsimd, ps=Tps)
            qk_mtile("q", 0, nc.gpsimd)
            qk_mtile("q", 1, nc.vector)

            # extras: v-projection fills head-0's window; the B-half w
            # packs and late QKV m-tiles (needed first by head 4) spread
            # thinly over the head-1/2 windows so their packs never crowd
            # the per-chunk DVE/Pool budget.
            def swB_pack(w):
                nc.vector.tensor_scalar(
                    swT[w][:, 0:4, AHG:HG], wstage[w][:, 0:4, AHG:HG],
                    0.0, 0.5, gt, sub,
                )
                nc.gpsimd.tensor_scalar(
                    swT[w][:, 4:8, AHG:HG], wstage[w][:, 4:8, AHG:HG],
                    0.0, 0.5, gt, sub,
                )

            extra_jobs = {}
            for t in range(4):
                extra_jobs[(0, (1, 3, 5, 6)[t])] = lambda t=t: v_ttile(t)
            for t in range(4, TC):
                extra_jobs[(1, t - 4)] = lambda t=t: v_ttile(
                    t, nc.vector if t < 6 else nc.gpsimd
                )
            extra_jobs[(2, 0)] = lambda: swB_pack("k")
            extra_jobs[(2, 2)] = lambda: swB_pack("q")
            extra_jobs[(2, 4)] = lambda: qk_mtile("q", 2, nc.vector)
            extra_jobs[(2, 6)] = lambda: qk_mtile("k", 2, nc.gpsimd)
            extra_jobs[(3, 0)] = lambda: qk_mtile("q", 3, nc.vector)
            extra_jobs[(3, 2)] = lambda: qk_mtile("k", 3, nc.gpsimd)

            # --- pipelined head loop ----------------------------------
            # Position schedule per window (head h), lag-1 for head h-1:
            #   c0,c1: scores->slots, pool exps of h-1 finishing up
            #   c1:    sum p3(h-1) + th(h-1)
            #   c2:    scores->Tps; pool compare c'=4 of h-1
            #   c3-c6: compares of h-1 (pool c'=5..7, DVE c'=0..3) + ctx
            #   c4,c5,c7: sums p0,p1,p2 of h
            #   c7:    output pack/DMA for a completed m-group
            for h in range(NH // G):
                open_head(h)
                for c in range(TC):
                    sc_exp(h, c)
                    if h >= 1 and c == 1:
                        fold_sum(h - 1, 3)
                        th(h - 1)
                    if c == 4:
                        fold_sum(h, 0)
                    elif c == 5:
                        fold_sum(h, 1)
                    elif c == 7:
                        fold_sum(h, 2)
                    job = extra_jobs.get((h, c))
                    if job is not None:
                        job()
                    if h >= 1:
                        hp_ = h - 1
                        if c == 2:
                            cmp_ctx(hp_, 4, 5, 3)
                        elif c == 3:
                            cmp_ctx(hp_, 5, 5, 3)
                            cmp_ctx(hp_, 0, 5, 3)
                        elif c == 4:
                            cmp_ctx(hp_, 6, 5, 3)
                            cmp_ctx(hp_, 1, 5, 3)
                        elif c == 5:
                            cmp_ctx(hp_, 7, 5, 3)
                            cmp_ctx(hp_, 2, 5, 3)
                        elif c == 6:
                            cmp_ctx(hp_, 3, 5, 3)
                        elif c == 7 and hp_ % 2 == 1:
                            out_m(hp_ // 2)
                del_h = h - 2
                if del_h in hstate:
                    del hstate[del_h]
            # tail: last head's sum/compare/context chain
            hlast = NH // G - 1
            fold_sum(hlast, 3, direct=True)
            th(hlast)
            for c in range(TC):
                cmp_ctx(hlast, c, 0, 7)
            out_m(MC - 1)
    return _split_multi_waits(nc)


_CACHE = {}


def _get_program(exp_scale, th_scale, out_scale, masked):
    key = (exp_scale, th_scale, out_scale, masked)
    if key not in _CACHE:
        _CACHE[key] = _build_program(exp_scale, th_scale, out_scale, masked)
    return _CACHE[key]


def make_in_maps(
    hidden_states,
    attention_mask,
    Wq,
    bq,
    Wk,
    bk,
    Wv,
    bv,
    a_q,
    a_k,
    a_v,
    clip_query,
    clip_key,
    clip_value,
    clip_attn,
):
    """Host-side marshalling: shard (pre-transposed, sign-preserving bf16
    delivery, q/k column fold-permutation) + fold scalar thresholds."""
    import ml_dtypes

    bf16 = ml_dtypes.bfloat16

    aq = max(float(np.asarray(a_q).reshape(-1)[0]), EPS)
    ak = max(float(np.asarray(a_k).reshape(-1)[0]), EPS)
    av = max(float(np.asarray(a_v).reshape(-1)[0]), EPS)
    cq = max(float(np.asarray(clip_query).reshape(-1)[0]), EPS)
    ck = max(float(np.asarray(clip_key).reshape(-1)[0]), EPS)
    cv = max(float(np.asarray(clip_value).reshape(-1)[0]), EPS)
    ca = max(float(np.asarray(clip_attn).reshape(-1)[0]), EPS)
    sq = float(np.abs(Wq).mean())
    sk = float(np.abs(Wk).mean())
    sv = float(np.abs(Wv).mean())

    # packed signs are +-0.5 -> matmul results are M/4
    thrq_full = (-bq / (4.0 * aq * sq)).astype(np.float32)
    thrk_full = (-bk / (4.0 * ak * sk)).astype(np.float32)
    bvrow_full = (bv / (4.0 * av * sv)).astype(np.float32)

    exp_scale = cq * ck * 0.5
    th_scale = 0.5 * ca
    out_scale = 2.0 * ca * cv

    mask = np.ascontiguousarray(
        np.asarray(attention_mask, dtype=np.float32).reshape(B, S)
    )
    masked = bool(np.any(mask != 0.0))

    hs = np.asarray(hidden_states, dtype=np.float32)
    hT = [np.ascontiguousarray(hs[b].T.astype(bf16)) for b in range(B)]
    WT = {
        "q": np.asarray(Wq, np.float32).T,
        "k": np.asarray(Wk, np.float32).T,
        "v": np.asarray(Wv, np.float32).T,
    }
    in_maps = []
    for core in range(NCORES):
        b, g = divmod(core, G)
        sl = slice(g * HG, (g + 1) * HG)
        wq_sh = WT["q"][:, sl]
        wk_sh = WT["k"][:, sl]
        in_maps.append(
            {
                "hT": hT[b],
                "wqT": np.ascontiguousarray(wq_sh[:, _PERM].astype(bf16)),
                "wkT": np.ascontiguousarray(wk_sh[:, _PERM].astype(bf16)),
                "wvT": np.ascontiguousarray(WT["v"][:, sl].astype(bf16)),
                "thrq": np.ascontiguousarray(thrq_full[sl][_PERM]),
                "thrk": np.ascontiguousarray(thrk_full[sl][_PERM]),
                "bvrow": np.ascontiguousarray(bvrow_full[sl]),
                "mask": mask[b],
            }
        )
    return in_maps, (exp_scale, th_scale, out_scale, masked)


def assemble_output(results):
    """Unshard: per-core ctxT [HG, S] -> [B, S, H] (transpose + concat)."""
    out = np.empty((B, S, H), dtype=np.float32)
    for core, res in enumerate(results):
        b, g = divmod(core, G)
        out[b, :, g * HG : (g + 1) * HG] = res["ctxT"].T
    return out


def kernel(**inputs) -> np.ndarray:
    from concourse.bass_utils import run_bass_kernel_spmd

    in_maps, scales = make_in_maps(**inputs)
    nc = _get_program(*scales)
    res = run_bass_kernel_spmd(nc, in_maps, list(range(NCORES)))
    return assemble_output(res.results)


# revision 16
# speedup vs baseline: 1.0031x; 1.0031x over previous
"""Trainium2 Bass kernel for binarized BERT self-attention (BiT-style).

Reference math:
  q = sign(h)*a_q @ (sign(Wq)*mean|Wq|).T + bq     (binarized linear)
  q2 = sign(q)*clip_q   (same for k, v)
  p  = softmax(q2 k2^T / sqrt(D) + mask)
  pq = clip(round(p/clip_a), 0, 1) * clip_a        (binary attention probs)
  out = pq @ v2

Exact algebraic facts used:
  * sign values pack as +-0.5 (exact in fp8e4m3/bf16); every matmul here is
    exact small-integer arithmetic accumulated in fp32 PSUM.
  * sign(x@W.T + b) thresholds the packed-sign matmul: (M/4) > -b/(4*a*s).
  * pq nonzero iff exp(s_i) > 0.5*clip_a * sum_j exp(s_j); scores bounded so
    no softmax max-pass is needed.  round() half-to-even matches strict '>'.

Sharding (8 cores): core = (batch b, head-group g); each core computes 8
heads x 1024 tokens and returns ctx^T [512 head-cols, 1024 tokens].

Performance structure (CoreSim cost-model driven, software-pipelined):
  * Scores run fp8 DoubleRow (0.5 cyc/row): the D=64 contraction is folded
    as [Ki=32 partitions, Ko=2]; Wq/Wk columns are host-permuted so each
    projection m-tile lands the fold layout directly.
  * exp() split across Scalar (chunks 0-6) and Pool (chunk 7) engines.
  * Softmax-sum matmul runs over pair-folded E (DVE folds) -> half PE work.
  * Prob compares: chunks 0-3 on DVE (bf16 P), chunks 4-7 on Pool writing
    fp8 P, so half the context matmul runs fp8 DoubleRow.
  * Emission is software-pipelined: head h's exp stream overlaps head h-1's
    compare/context tail, V-projection and late QKV tiles fill the first
    two head windows, and all DMAs are batched with the critical h/wq/wk
    bytes front-loaded (bf16 delivery, sign-preserving).
"""

import math

import numpy as np

B, S, H, NH, D = 4, 1024, 1024, 16, 64
NCORES, G = 8, 2
HG = H // G  # 512 output columns per core (8 heads)
EPS = 1e-5
KC = H // 128  # 8 contraction chunks
TC = S // 128  # 8 token chunks
MC = HG // 128  # 4 output chunks per core


def _qk_perm():
    """Column order for Wq/Wk shards: m-tile m=(2*hq+o), partition p=32*b+r
    holds neuron (4*hq+b)*64 + o*32 + r, so the projection psum directly
    yields the [Ki=32, Ko=2] DoubleRow fold for the scores contraction."""
    perm = np.empty(HG, dtype=np.int64)
    for m in range(MC):
        hq, o = m // 2, m % 2
        for p in range(128):
            b, r = p // 32, p % 32
            perm[m * 128 + p] = (4 * hq + b) * 64 + o * 32 + r
    return perm


_PERM = _qk_perm()


def _split_multi_waits(nc):
    """Walrus accepts at most ONE sync-wait per instruction.  Move all but
    one wait onto preceding same-engine NOPs (semantically equivalent since
    engines execute their streams in order)."""
    from concourse import mybir

    eng_api = {
        mybir.EngineType.PE: nc.tensor,
        mybir.EngineType.DVE: nc.vector,
        mybir.EngineType.Activation: nc.scalar,
        mybir.EngineType.Pool: nc.gpsimd,
        mybir.EngineType.SP: nc.sync,
    }

    plan = []
    for f in nc.m.functions:
        for bb in f.blocks:
            for ins in bb.instructions:
                si = ins.sync_info
                if si is None or not si.on_wait or len(si.on_wait) <= 1:
                    continue
                plan.append((f, bb, ins))

    fillers = {}
    for f, bb, ins in plan:
        si = ins.sync_info
        waits = list(si.on_wait)
        nops = []
        for w in waits[:-1]:
            bi = eng_api[ins.engine].nop()
            raw = bi.ins
            raw.sync_info = mybir.SyncInfo(on_wait=[w], on_update=[])
            nops.append(raw)
        ins.sync_info = mybir.SyncInfo(
            on_wait=[waits[-1]], on_update=list(si.on_update or [])
        )
        fillers[ins.name] = nops

    created = {n.name for nops in fillers.values() for n in nops}
    for f in nc.m.functions:
        for bb in f.blocks:
            out = []
            for ins in bb.instructions:
                if ins.name in created:
                    continue
                out.extend(fillers.get(ins.name, ()))
                out.append(ins)
            bb.instructions = out
    return nc


def _build_program(exp_scale: float, th_scale: float, out_scale: float,
                   masked: bool):
    import concourse.bass as bass
    import concourse.tile as tile
    from concourse import mybir

    f32, bf16 = mybir.dt.float32, mybir.dt.bfloat16
    fp8 = mybir.dt.float8e4
    DR = mybir.MatmulPerfMode.DoubleRow
    gt = mybir.AluOpType.is_gt
    sub = mybir.AluOpType.subtract
    mult = mybir.AluOpType.mult
    add = mybir.AluOpType.add
    Exp = mybir.ActivationFunctionType.Exp

    nc = bass.Bass()
    hT_d = nc.dram_tensor("hT", [H, S], bf16, kind="ExternalInput")
    wT_d = {
        w: nc.dram_tensor(f"w{w}T", [H, HG], bf16, kind="ExternalInput")
        for w in "qkv"
    }
    thrq_d = nc.dram_tensor("thrq", [HG], f32, kind="ExternalInput")
    thrk_d = nc.dram_tensor("thrk", [HG], f32, kind="ExternalInput")
    bvrow_d = nc.dram_tensor("bvrow", [HG], f32, kind="ExternalInput")
    mask_d = nc.dram_tensor("mask", [S], f32, kind="ExternalInput")
    out_d = nc.dram_tensor("ctxT", [HG, S], f32, kind="ExternalOutput")

    with tile.TileContext(nc) as tc:
        with (
            tc.tile_pool(name="persist", bufs=1) as persist,
            tc.tile_pool(name="heads", bufs=2) as headp,
            tc.tile_pool(name="ps_s", bufs=2, space="PSUM") as ps_s,
            tc.tile_pool(name="ps_t", bufs=1, space="PSUM") as ps_t,
            tc.tile_pool(name="ps_c", bufs=1, space="PSUM") as ps_c,
        ):
            hstage = persist.tile([128, KC, S], bf16, tag="hstage")
            wstage = {
                w: persist.tile(
                    [128, KC, HG], bf16, tag=f"wstage_{w}", name=f"wstage_{w}"
                )
                for w in "qkv"
            }
            shT = persist.tile([128, KC, S], fp8, tag="shT")
            swT = {
                w: persist.tile(
                    [128, KC, HG], fp8, tag=f"swT_{w}", name=f"swT_{w}"
                )
                for w in "qkv"
            }
            # fold layout: [32*b + r, hq, o, token]; head 4*hq+b, d = o*32+r
            qT4 = persist.tile([128, 2, 2, S], fp8, tag="qT4")
            kT4 = persist.tile([128, 2, 2, S], fp8, tag="kT4")
            v_bf = persist.tile([128, 4, HG], bf16, tag="v_bf")  # t=0..3
            v_q8 = persist.tile([128, 4, HG], fp8, tag="v_q8")  # t=4..7
            thrq_sb = persist.tile([128, MC], f32, tag="thrq")
            thrk_sb = persist.tile([128, MC], f32, tag="thrk")
            bvrow_sb = persist.tile([1, HG], f32, tag="bvrow")
            bvrow16 = persist.tile([1, HG], bf16, tag="bvrow16")
            mask_sb = persist.tile([128, TC], f32, tag="mask")
            ones16 = persist.tile([1, 128], bf16, tag="ones16")
            onesK = persist.tile([128, 128], bf16, tag="onesK")
            out_sb = persist.tile([128, MC, S], f32, tag="out_sb")

            # score/projection psum comes from a 2-buffer pool (2 banks per
            # buffer -> 4 banks total); Tps/Cps are persistent single tiles.
            Tps = ps_t.tile([128, S], f32, tag="Tps")  # 2 banks
            Cps = ps_c.tile([128, S], f32, tag="Cps")  # 2 banks

            # warmup: memsets on DVE, then dummy matmuls keep the PE
            # continuously busy through the load phase so the p-state ramp
            # completes before the first real projection; a dummy exp
            # pre-loads the activation table off the critical path.
            nc.vector.memset(onesK, 1.0)
            nc.vector.memset(ones16, 1.0)
            warm_sb = persist.tile([1, 128], bf16, tag="warm_sb")
            nc.scalar.activation(warm_sb, ones16, Exp, bias=0.0, scale=0.0)
            for _ in range(120):
                nc.tensor.matmul(
                    Tps[:, 0:128], lhsT=onesK, rhs=onesK,
                    start=True, stop=True,
                )
            # tiny control tensors first so their (negligible) transfers
            # clear the DMA device before the bulk loads queue up.
            nc.gpsimd.dma_start(
                out=thrq_sb, in_=thrq_d.rearrange("(m p) -> p m", p=128)
            )
            nc.gpsimd.dma_start(
                out=thrk_sb, in_=thrk_d.rearrange("(m p) -> p m", p=128)
            )
            nc.gpsimd.dma_start(
                out=bvrow_sb, in_=bvrow_d.rearrange("(o n) -> o n", o=1)
            )
            nc.gpsimd.memset(mask_sb, 0.0)
            if masked:
                nc.gpsimd.dma_start(
                    out=mask_sb, in_=mask_d.rearrange("(t p) -> p t", p=128)
                )
            nc.vector.tensor_copy(bvrow16, bvrow_sb)

            # --- input DMAs.  All bulk loads issue on SP in strict
            # priority order (h quarters, then the m0/m1 halves of wq/wk,
            # then the rest) -- a single issuer guarantees DMA-device order
            # so nothing steals slots from the critical path.
            AHG = HG // 2  # 256: columns of m-tiles 0,1
            for qtr in range(4):
                nc.sync.dma_start(
                    out=hstage[:, 2 * qtr : 2 * qtr + 2, :],
                    in_=hT_d[qtr * 256 : (qtr + 1) * 256, :].rearrange(
                        "(c p) s -> p c s", p=128
                    ),
                )
                eng = nc.vector if qtr % 2 == 0 else nc.gpsimd
                eng.tensor_scalar(
                    shT[:, 2 * qtr : 2 * qtr + 2, :],
                    hstage[:, 2 * qtr : 2 * qtr + 2, :],
                    0.0,
                    0.5,
                    gt,
                    sub,
                )
            for w in ("k", "q"):
                nc.sync.dma_start(
                    out=wstage[w][:, :, 0:AHG],
                    in_=wT_d[w][:, 0:AHG].rearrange("(c p) o -> p c o", p=128),
                )
                nc.vector.tensor_scalar(
                    swT[w][:, :, 0:AHG], wstage[w][:, :, 0:AHG],
                    0.0, 0.5, gt, sub,
                )
            for w in ("k", "q"):
                nc.sync.dma_start(
                    out=wstage[w][:, :, AHG:HG],
                    in_=wT_d[w][:, AHG:HG].rearrange(
                        "(c p) o -> p c o", p=128
                    ),
                )
            nc.sync.dma_start(
                out=wstage["v"],
                in_=wT_d["v"].rearrange("(c p) o -> p c o", p=128),
            )
            nc.vector.tensor_scalar(
                swT["v"], wstage["v"], 0.0, 0.5, gt, sub
            )

            # --- emission helpers -------------------------------------
            # score/projection psum tiles cycle through the 2-buffer pool;
            # WAR hazards between consecutive users are tracked per tile.
            slot_counter = [0]

            def next_slot():
                slot_counter[0] += 1
                return ps_s.tile(
                    [128, S], f32, tag="S", name=f"S{slot_counter[0]}"
                )

            def qk_mtile(w, m, pack_eng, ps=None):
                """Q/K projection m-tile into a psum slot, pack to fp8."""
                dstT = qT4 if w == "q" else kT4
                thr = thrq_sb if w == "q" else thrk_sb
                if ps is None:
                    ps = next_slot()
                for half in range(2):
                    sl = slice(half * 512, (half + 1) * 512)
                    for c2 in range(KC // 2):
                        nc.tensor.matmul(
                            ps[:, sl],
                            lhsT=swT[w][
                                :, 2 * c2 : 2 * c2 + 2, m * 128 : (m + 1) * 128
                            ],
                            rhs=shT[:, 2 * c2 : 2 * c2 + 2, sl],
                            start=(c2 == 0),
                            stop=(c2 == KC // 2 - 1),
                            perf_mode=DR,
                        )
                pack_eng.tensor_scalar(
                    dstT[:, m // 2, m % 2, :], ps, thr[:, m : m + 1], 0.5,
                    gt, sub,
                )

            def v_ttile(t, pack_eng=None):
                """V projection token-tile via a Cps half-slot; bf16 for
                t<4 (bf16 ctx chunks), fp8 for t>=4 (DoubleRow ctx pairs)."""
                ps = Cps[:, (t % 2) * 512 : (t % 2 + 1) * 512]
                for c2 in range(KC // 2):
                    nc.tensor.matmul(
                        ps,
                        lhsT=shT[
                            :, 2 * c2 : 2 * c2 + 2, t * 128 : (t + 1) * 128
                        ],
                        rhs=swT["v"][:, 2 * c2 : 2 * c2 + 2, :],
                        start=(c2 == 0),
                        stop=False,
                        perf_mode=DR,
                    )
                nc.tensor.matmul(
                    ps, lhsT=ones16, rhs=bvrow16, start=False, stop=True
                )
                dst = v_bf[:, t, :] if t < 4 else v_q8[:, t - 4, :]
                (pack_eng or nc.gpsimd).tensor_scalar(
                    dst, ps, 0.0, 0.5, gt, sub
                )

            # head-local state for the pipelined tail
            hstate = {}

            def sc_exp(h, c):
                st = hstate[h]
                # chunk 2 borrows the Tps banks (free between th(h-1) and
                # this window's first sum) -- a third slot that breaks the
                # 2-slot WAR chain pacing the activation stream.
                ps = Tps if c == 2 else next_slot()
                kv, qv = st["kv"], st["qv"]
                for sp in range(2):
                    sl = slice(sp * 512, (sp + 1) * 512)
                    nc.tensor.matmul(
                        ps[:, sl],
                        lhsT=kv[:, :, c * 128 : (c + 1) * 128],
                        rhs=qv[:, :, sl],
                        start=True,
                        stop=True,
                        perf_mode=DR,
                        tile_position=(32 * st["b"], 0),
                    )
                if c < 6 or h == NH // G - 1:
                    nc.scalar.activation(
                        st["E"][:, c, :], ps, Exp,
                        bias=mask_sb[:, c : c + 1], scale=exp_scale,
                    )
                else:
                    eeng = nc.gpsimd
                    eeng.add_instruction(
                        mybir.InstActivation(
                            name=nc.get_next_instruction_name(),
                            func=Exp,
                            ins=[
                                eeng.lower_ap(ps),
                                eeng.lower_ap(mask_sb[:, c : c + 1]),
                                mybir.ImmediateValue(
                                    dtype=f32, value=exp_scale
                                ),
                                mybir.ImmediateValue(dtype=f32, value=0.0),
                            ],
                            outs=[eeng.lower_ap(st["E"][:, c, :])],
                        )
                    )

            def fold_sum(h, p, direct=False):
                st = hstate[h]
                if direct:
                    # skip the fold: sum the two raw chunks (shorter latency
                    # for the final head's tail at slightly more PE work)
                    for ci in range(2):
                        for sp in range(2):
                            sl = slice(sp * 512, (sp + 1) * 512)
                            nc.tensor.matmul(
                                Tps[:, sl],
                                lhsT=onesK,
                                rhs=st["E"][:, 2 * p + ci, sl],
                                start=(p == 0 and ci == 0),
                                stop=(p == TC // 2 - 1 and ci == 1),
                            )
                    return
                nc.vector.tensor_tensor(
                    st["Efold"][:, p, :],
                    st["E"][:, 2 * p, :],
                    st["E"][:, 2 * p + 1, :],
                    add,
                )
                for sp in range(2):
                    sl = slice(sp * 512, (sp + 1) * 512)
                    nc.tensor.matmul(
                        Tps[:, sl],
                        lhsT=onesK,
                        rhs=st["Efold"][:, p, sl],
                        start=(p == 0),
                        stop=(p == TC // 2 - 1),
                    )

            def th(h):
                st = hstate[h]
                # halves on DVE and Pool in parallel to halve the latency
                nc.vector.tensor_scalar(
                    st["Th"][:, 0:512], Tps[:, 0:512], th_scale, None, mult
                )
                nc.gpsimd.tensor_scalar(
                    st["Th"][:, 512:1024], Tps[:, 512:1024],
                    th_scale, None, mult,
                )

            def cmp_ctx(h, c, start_c, stop_c):
                st = hstate[h]
                hl, hp = st["hl"], st["hp"]
                if c < 4:
                    nc.vector.tensor_tensor(
                        st["P"][:, c, :], st["E"][:, c, :], st["Th"], gt
                    )
                    for sp in range(2):
                        sl = slice(sp * 512, (sp + 1) * 512)
                        nc.tensor.matmul(
                            Cps[hp : hp + 64, sl],
                            lhsT=v_bf[:, c, hl * 64 : (hl + 1) * 64],
                            rhs=st["P"][:, c, sl],
                            start=(c == start_c),
                            stop=(c == stop_c),
                            tile_position=(0, hp),
                        )
                else:
                    nc.gpsimd.tensor_tensor(
                        st["Pq"][:, c - 4, :], st["E"][:, c, :], st["Th"], gt
                    )
                    if c in (5, 7):  # DoubleRow pair (c-1, c)
                        pr = (c - 5) // 2  # 0 or 1
                        for sp in range(2):
                            sl = slice(sp * 512, (sp + 1) * 512)
                            nc.tensor.matmul(
                                Cps[hp : hp + 64, sl],
                                lhsT=v_q8[
                                    :, 2 * pr : 2 * pr + 2,
                                    hl * 64 : (hl + 1) * 64,
                                ],
                                rhs=st["Pq"][:, 2 * pr : 2 * pr + 2, sl],
                                start=(c == start_c),
                                stop=(c == stop_c),
                                perf_mode=DR,
                                tile_position=(0, hp),
                            )

            def out_m(m):
                for half in range(2):
                    sl = slice(half * 512, (half + 1) * 512)
                    eng = nc.gpsimd if half == 0 else nc.vector
                    eng.tensor_scalar(
                        out_sb[:, m, sl], Cps[:, sl], out_scale, None, mult
                    )
                    nc.sync.dma_start(
                        out=out_d.rearrange("(m p) s -> p m s", p=128)[
                            :, m, sl
                        ],
                        in_=out_sb[:, m, sl],
                    )

            def open_head(h):
                hl = h
                b, hq = hl % 4, hl // 4
                hstate[h] = {
                    "hl": hl,
                    "b": b,
                    "hp": 64 * (hl % 2),
                    "kv": kT4[32 * b : 32 * b + 32, hq, :, :],
                    "qv": qT4[32 * b : 32 * b + 32, hq, :, :],
                    "E": headp.tile([128, TC, S], bf16, tag="E", name=f"E_{h}"),
                    "Efold": headp.tile(
                        [128, TC // 2, S], bf16, tag="Ef", name=f"Ef_{h}"
                    ),
                    "P": headp.tile([128, 4, S], bf16, tag="P", name=f"P_{h}"),
                    "Pq": headp.tile(
                        [128, 4, S], fp8, tag="Pq", name=f"Pq_{h}"
                    ),
                    "Th": headp.tile([128, S], bf16, tag="Th", name=f"Th_{h}"),
                }

            # --- prelude QKV: m-tiles 0,1 of k then q (enough for heads
            # 0-3); k1 borrows the Tps banks so all four tiles pipeline
            # without slot-reuse stalls; packs split across DVE and Pool.
            qk_mtile("k", 0, nc.vector)
            qk_mtile("k", 1, nc.gpsimd, ps=Tps)
            qk_mtile("q", 0, nc.gpsimd)
            qk_mtile("q", 1, nc.vector)

            # extras: v-projection fills head-0's window; the B-half w
            # packs and late QKV m-tiles (needed first by head 4) spread
            # thinly over the head-1/2 windows so their packs never crowd
            # the per-chunk DVE/Pool budget.
            def swB_pack(w):
                nc.vector.tensor_scalar(
                    swT[w][:, 0:4, AHG:HG], wstage[w][:, 0:4, AHG:HG],
                    0.0, 0.5, gt, sub,
                )
                nc.gpsimd.tensor_scalar(
                    swT[w][:, 4:8, AHG:HG], wstage[w][:, 4:8, AHG:HG],
                    0.0, 0.5, gt, sub,
                )

            extra_jobs = {}
            for t in range(4):
                extra_jobs[(0, (1, 3, 5, 6)[t])] = lambda t=t: v_ttile(t)
            for t in range(4, TC):
                extra_jobs[(1, t - 4)] = lambda t=t: v_ttile(
                    t, nc.vector if t < 6 else nc.gpsimd
                )
            extra_jobs[(2, 0)] = lambda: swB_pack("k")
            extra_jobs[(2, 2)] = lambda: swB_pack("q")
            extra_jobs[(2, 4)] = lambda: qk_mtile("q", 2, nc.vector)
            extra_jobs[(2, 6)] = lambda: qk_mtile("k", 2, nc.gpsimd)
            extra_jobs[(3, 0)] = lambda: qk_mtile("q", 3, nc.vector)
            extra_jobs[(3, 2)] = lambda: qk_mtile("k", 3, nc.gpsimd)

            # --- pipelined head loop ----------------------------------
            # Position schedule per window (head h), lag-1 for head h-1:
            #   c0,c1: scores->slots, pool exps of h-1 finishing up
            #   c1:    sum p3(h-1) + th(h-1)
            #   c2:    scores->Tps; pool compare c'=4 of h-1
            #   c3-c6: compares of h-1 (pool c'=5..7, DVE c'=0..3) + ctx
            #   c4,c5,c7: sums p0,p1,p2 of h
            #   c7:    output pack/DMA for a completed m-group
            for h in range(NH // G):
                open_head(h)
                for c in range(TC):
                    sc_exp(h, c)
                    if h >= 1 and c == 1:
                        fold_sum(h - 1, 3)
                        th(h - 1)
                    if c == 4:
                        fold_sum(h, 0)
                    elif c == 6:
                        fold_sum(h, 1)
                    elif c == 7:
                        fold_sum(h, 2)
                    job = extra_jobs.get((h, c))
                    if job is not None:
                        job()
                    if h >= 1:
                        hp_ = h - 1
                        if c == 2:
                            cmp_ctx(hp_, 4, 5, 3)
                        elif c == 3:
                            cmp_ctx(hp_, 5, 5, 3)
                            cmp_ctx(hp_, 0, 5, 3)
                        elif c == 4:
                            cmp_ctx(hp_, 6, 5, 3)
                            cmp_ctx(hp_, 1, 5, 3)
                        elif c == 5:
                            cmp_ctx(hp_, 7, 5, 3)
                            cmp_ctx(hp_, 2, 5, 3)
                        elif c == 6:
                            cmp_ctx(hp_, 3, 5, 3)
                        elif c == 7 and hp_ % 2 == 1:
                            out_m(hp_ // 2)
                del_h = h - 2
                if del_h in hstate:
                    del hstate[del_h]
            # tail: last head's sum/compare/context chain
            hlast = NH // G - 1
            fold_sum(hlast, 3, direct=True)
            th(hlast)
            for c in range(TC):
                cmp_ctx(hlast, c, 0, 7)
            out_m(MC - 1)
    return _split_multi_waits(nc)


_CACHE = {}


def _get_program(exp_scale, th_scale, out_scale, masked):
    key = (exp_scale, th_scale, out_scale, masked)
    if key not in _CACHE:
        _CACHE[key] = _build_program(exp_scale, th_scale, out_scale, masked)
    return _CACHE[key]


def make_in_maps(
    hidden_states,
    attention_mask,
    Wq,
    bq,
    Wk,
    bk,
    Wv,
    bv,
    a_q,
    a_k,
    a_v,
    clip_query,
    clip_key,
    clip_value,
    clip_attn,
):
    """Host-side marshalling: shard (pre-transposed, sign-preserving bf16
    delivery, q/k column fold-permutation) + fold scalar thresholds."""
    import ml_dtypes

    bf16 = ml_dtypes.bfloat16

    aq = max(float(np.asarray(a_q).reshape(-1)[0]), EPS)
    ak = max(float(np.asarray(a_k).reshape(-1)[0]), EPS)
    av = max(float(np.asarray(a_v).reshape(-1)[0]), EPS)
    cq = max(float(np.asarray(clip_query).reshape(-1)[0]), EPS)
    ck = max(float(np.asarray(clip_key).reshape(-1)[0]), EPS)
    cv = max(float(np.asarray(clip_value).reshape(-1)[0]), EPS)
    ca = max(float(np.asarray(clip_attn).reshape(-1)[0]), EPS)
    sq = float(np.abs(Wq).mean())
    sk = float(np.abs(Wk).mean())
    sv = float(np.abs(Wv).mean())

    # packed signs are +-0.5 -> matmul results are M/4
    thrq_full = (-bq / (4.0 * aq * sq)).astype(np.float32)
    thrk_full = (-bk / (4.0 * ak * sk)).astype(np.float32)
    bvrow_full = (bv / (4.0 * av * sv)).astype(np.float32)

    exp_scale = cq * ck * 0.5
    th_scale = 0.5 * ca
    out_scale = 2.0 * ca * cv

    mask = np.ascontiguousarray(
        np.asarray(attention_mask, dtype=np.float32).reshape(B, S)
    )
    masked = bool(np.any(mask != 0.0))

    hs = np.asarray(hidden_states, dtype=np.float32)
    hT = [np.ascontiguousarray(hs[b].T.astype(bf16)) for b in range(B)]
    WT = {
        "q": np.asarray(Wq, np.float32).T,
        "k": np.asarray(Wk, np.float32).T,
        "v": np.asarray(Wv, np.float32).T,
    }
    in_maps = []
    for core in range(NCORES):
        b, g = divmod(core, G)
        sl = slice(g * HG, (g + 1) * HG)
        wq_sh = WT["q"][:, sl]
        wk_sh = WT["k"][:, sl]
        in_maps.append(
            {
                "hT": hT[b],
                "wqT": np.ascontiguousarray(wq_sh[:, _PERM].astype(bf16)),
                "wkT": np.ascontiguousarray(wk_sh[:, _PERM].astype(bf16)),
                "wvT": np.ascontiguousarray(WT["v"][:, sl].astype(bf16)),
                "thrq": np.ascontiguousarray(thrq_full[sl][_PERM]),
                "thrk": np.ascontiguousarray(thrk_full[sl][_PERM]),
                "bvrow": np.ascontiguousarray(bvrow_full[sl]),
                "mask": mask[b],
            }
        )
    return in_maps, (exp_scale, th_scale, out_scale, masked)


def assemble_output(results):
    """Unshard: per-core ctxT [HG, S] -> [B, S, H] (transpose + concat)."""
    out = np.empty((B, S, H), dtype=np.float32)
    for core, res in enumerate(results):
        b, g = divmod(core, G)
        out[b, :, g * HG : (g + 1) * HG] = res["ctxT"].T
    return out


def kernel(**inputs) -> np.ndarray:
    from concourse.bass_utils import run_bass_kernel_spmd

    in_maps, scales = make_in_maps(**inputs)
    nc = _get_program(*scales)
    res = run_bass_kernel_spmd(nc, in_maps, list(range(NCORES)))
    return assemble_output(res.results)


# revision 17
# speedup vs baseline: 1.0039x; 1.0007x over previous
"""Trainium2 Bass kernel for binarized BERT self-attention (BiT-style).

Reference math:
  q = sign(h)*a_q @ (sign(Wq)*mean|Wq|).T + bq     (binarized linear)
  q2 = sign(q)*clip_q   (same for k, v)
  p  = softmax(q2 k2^T / sqrt(D) + mask)
  pq = clip(round(p/clip_a), 0, 1) * clip_a        (binary attention probs)
  out = pq @ v2

Exact algebraic facts used:
  * sign values pack as +-0.5 (exact in fp8e4m3/bf16); every matmul here is
    exact small-integer arithmetic accumulated in fp32 PSUM.
  * sign(x@W.T + b) thresholds the packed-sign matmul: (M/4) > -b/(4*a*s).
  * pq nonzero iff exp(s_i) > 0.5*clip_a * sum_j exp(s_j); scores bounded so
    no softmax max-pass is needed.  round() half-to-even matches strict '>'.

Sharding (8 cores): core = (batch b, head-group g); each core computes 8
heads x 1024 tokens and returns ctx^T [512 head-cols, 1024 tokens].

Performance structure (CoreSim cost-model driven, software-pipelined):
  * Scores run fp8 DoubleRow (0.5 cyc/row): the D=64 contraction is folded
    as [Ki=32 partitions, Ko=2]; Wq/Wk columns are host-permuted so each
    projection m-tile lands the fold layout directly.
  * exp() split across Scalar (chunks 0-6) and Pool (chunk 7) engines.
  * Softmax-sum matmul runs over pair-folded E (DVE folds) -> half PE work.
  * Prob compares: chunks 0-3 on DVE (bf16 P), chunks 4-7 on Pool writing
    fp8 P, so half the context matmul runs fp8 DoubleRow.
  * Emission is software-pipelined: head h's exp stream overlaps head h-1's
    compare/context tail, V-projection and late QKV tiles fill the first
    two head windows, and all DMAs are batched with the critical h/wq/wk
    bytes front-loaded (bf16 delivery, sign-preserving).
"""

import math

import numpy as np

B, S, H, NH, D = 4, 1024, 1024, 16, 64
NCORES, G = 8, 2
HG = H // G  # 512 output columns per core (8 heads)
EPS = 1e-5
KC = H // 128  # 8 contraction chunks
TC = S // 128  # 8 token chunks
MC = HG // 128  # 4 output chunks per core


def _qk_perm():
    """Column order for Wq/Wk shards: m-tile m=(2*hq+o), partition p=32*b+r
    holds neuron (4*hq+b)*64 + o*32 + r, so the projection psum directly
    yields the [Ki=32, Ko=2] DoubleRow fold for the scores contraction."""
    perm = np.empty(HG, dtype=np.int64)
    for m in range(MC):
        hq, o = m // 2, m % 2
        for p in range(128):
            b, r = p // 32, p % 32
            perm[m * 128 + p] = (4 * hq + b) * 64 + o * 32 + r
    return perm


_PERM = _qk_perm()


def _split_multi_waits(nc):
    """Walrus accepts at most ONE sync-wait per instruction.  Move all but
    one wait onto preceding same-engine NOPs (semantically equivalent since
    engines execute their streams in order)."""
    from concourse import mybir

    eng_api = {
        mybir.EngineType.PE: nc.tensor,
        mybir.EngineType.DVE: nc.vector,
        mybir.EngineType.Activation: nc.scalar,
        mybir.EngineType.Pool: nc.gpsimd,
        mybir.EngineType.SP: nc.sync,
    }

    plan = []
    for f in nc.m.functions:
        for bb in f.blocks:
            for ins in bb.instructions:
                si = ins.sync_info
                if si is None or not si.on_wait or len(si.on_wait) <= 1:
                    continue
                plan.append((f, bb, ins))

    fillers = {}
    for f, bb, ins in plan:
        si = ins.sync_info
        waits = list(si.on_wait)
        nops = []
        for w in waits[:-1]:
            bi = eng_api[ins.engine].nop()
            raw = bi.ins
            raw.sync_info = mybir.SyncInfo(on_wait=[w], on_update=[])
            nops.append(raw)
        ins.sync_info = mybir.SyncInfo(
            on_wait=[waits[-1]], on_update=list(si.on_update or [])
        )
        fillers[ins.name] = nops

    created = {n.name for nops in fillers.values() for n in nops}
    for f in nc.m.functions:
        for bb in f.blocks:
            out = []
            for ins in bb.instructions:
                if ins.name in created:
                    continue
                out.extend(fillers.get(ins.name, ()))
                out.append(ins)
            bb.instructions = out
    return nc


def _build_program(exp_scale: float, th_scale: float, out_scale: float,
                   masked: bool):
    import concourse.bass as bass
    import concourse.tile as tile
    from concourse import mybir

    f32, bf16 = mybir.dt.float32, mybir.dt.bfloat16
    fp8 = mybir.dt.float8e4
    DR = mybir.MatmulPerfMode.DoubleRow
    gt = mybir.AluOpType.is_gt
    sub = mybir.AluOpType.subtract
    mult = mybir.AluOpType.mult
    add = mybir.AluOpType.add
    Exp = mybir.ActivationFunctionType.Exp

    nc = bass.Bass()
    hT_d = nc.dram_tensor("hT", [H, S], bf16, kind="ExternalInput")
    wT_d = {
        w: nc.dram_tensor(f"w{w}T", [H, HG], bf16, kind="ExternalInput")
        for w in "qkv"
    }
    thrq_d = nc.dram_tensor("thrq", [HG], f32, kind="ExternalInput")
    thrk_d = nc.dram_tensor("thrk", [HG], f32, kind="ExternalInput")
    bvrow_d = nc.dram_tensor("bvrow", [HG], f32, kind="ExternalInput")
    mask_d = nc.dram_tensor("mask", [S], f32, kind="ExternalInput")
    out_d = nc.dram_tensor("ctxT", [HG, S], f32, kind="ExternalOutput")

    with tile.TileContext(nc) as tc:
        with (
            tc.tile_pool(name="persist", bufs=1) as persist,
            tc.tile_pool(name="heads", bufs=2) as headp,
            tc.tile_pool(name="ps_s", bufs=2, space="PSUM") as ps_s,
            tc.tile_pool(name="ps_t", bufs=1, space="PSUM") as ps_t,
            tc.tile_pool(name="ps_c", bufs=1, space="PSUM") as ps_c,
        ):
            hstage = persist.tile([128, KC, S], bf16, tag="hstage")
            wstage = {
                w: persist.tile(
                    [128, KC, HG], bf16, tag=f"wstage_{w}", name=f"wstage_{w}"
                )
                for w in "qkv"
            }
            shT = persist.tile([128, KC, S], fp8, tag="shT")
            swT = {
                w: persist.tile(
                    [128, KC, HG], fp8, tag=f"swT_{w}", name=f"swT_{w}"
                )
                for w in "qkv"
            }
            # fold layout: [32*b + r, hq, o, token]; head 4*hq+b, d = o*32+r
            qT4 = persist.tile([128, 2, 2, S], fp8, tag="qT4")
            kT4 = persist.tile([128, 2, 2, S], fp8, tag="kT4")
            v_bf = persist.tile([128, 4, HG], bf16, tag="v_bf")  # t=0..3
            v_q8 = persist.tile([128, 4, HG], fp8, tag="v_q8")  # t=4..7
            thrq_sb = persist.tile([128, MC], f32, tag="thrq")
            thrk_sb = persist.tile([128, MC], f32, tag="thrk")
            bvrow_sb = persist.tile([1, HG], f32, tag="bvrow")
            bvrow16 = persist.tile([1, HG], bf16, tag="bvrow16")
            mask_sb = persist.tile([128, TC], f32, tag="mask")
            ones16 = persist.tile([1, 128], bf16, tag="ones16")
            onesK = persist.tile([128, 128], bf16, tag="onesK")
            out_sb = persist.tile([128, MC, S], f32, tag="out_sb")

            # score/projection psum comes from a 2-buffer pool (2 banks per
            # buffer -> 4 banks total); Tps/Cps are persistent single tiles.
            Tps = ps_t.tile([128, S], f32, tag="Tps")  # 2 banks
            Cps = ps_c.tile([128, S], f32, tag="Cps")  # 2 banks

            # warmup: memsets on DVE, then dummy matmuls keep the PE
            # continuously busy through the load phase so the p-state ramp
            # completes before the first real projection; a dummy exp
            # pre-loads the activation table off the critical path.
            nc.vector.memset(onesK, 1.0)
            nc.vector.memset(ones16, 1.0)
            warm_sb = persist.tile([1, 128], bf16, tag="warm_sb")
            nc.scalar.activation(warm_sb, ones16, Exp, bias=0.0, scale=0.0)
            for _ in range(120):
                nc.tensor.matmul(
                    Tps[:, 0:128], lhsT=onesK, rhs=onesK,
                    start=True, stop=True,
                )
            # tiny control tensors first so their (negligible) transfers
            # clear the DMA device before the bulk loads queue up.
            nc.gpsimd.dma_start(
                out=thrq_sb, in_=thrq_d.rearrange("(m p) -> p m", p=128)
            )
            nc.gpsimd.dma_start(
                out=thrk_sb, in_=thrk_d.rearrange("(m p) -> p m", p=128)
            )
            nc.gpsimd.dma_start(
                out=bvrow_sb, in_=bvrow_d.rearrange("(o n) -> o n", o=1)
            )
            nc.gpsimd.memset(mask_sb, 0.0)
            if masked:
                nc.gpsimd.dma_start(
                    out=mask_sb, in_=mask_d.rearrange("(t p) -> p t", p=128)
                )
            nc.vector.tensor_copy(bvrow16, bvrow_sb)

            # --- input DMAs.  All bulk loads issue on SP in strict
            # priority order (h quarters, then the m0/m1 halves of wq/wk,
            # then the rest) -- a single issuer guarantees DMA-device order
            # so nothing steals slots from the critical path.
            AHG = HG // 2  # 256: columns of m-tiles 0,1
            for qtr in range(4):
                nc.sync.dma_start(
                    out=hstage[:, 2 * qtr : 2 * qtr + 2, :],
                    in_=hT_d[qtr * 256 : (qtr + 1) * 256, :].rearrange(
                        "(c p) s -> p c s", p=128
                    ),
                )
                eng = nc.vector if qtr % 2 == 0 else nc.gpsimd
                eng.tensor_scalar(
                    shT[:, 2 * qtr : 2 * qtr + 2, :],
                    hstage[:, 2 * qtr : 2 * qtr + 2, :],
                    0.0,
                    0.5,
                    gt,
                    sub,
                )
            for w in ("k", "q"):
                nc.sync.dma_start(
                    out=wstage[w][:, :, 0:AHG],
                    in_=wT_d[w][:, 0:AHG].rearrange("(c p) o -> p c o", p=128),
                )
                nc.vector.tensor_scalar(
                    swT[w][:, :, 0:AHG], wstage[w][:, :, 0:AHG],
                    0.0, 0.5, gt, sub,
                )
            for w in ("k", "q"):
                nc.sync.dma_start(
                    out=wstage[w][:, :, AHG:HG],
                    in_=wT_d[w][:, AHG:HG].rearrange(
                        "(c p) o -> p c o", p=128
                    ),
                )
            nc.sync.dma_start(
                out=wstage["v"],
                in_=wT_d["v"].rearrange("(c p) o -> p c o", p=128),
            )
            nc.vector.tensor_scalar(
                swT["v"], wstage["v"], 0.0, 0.5, gt, sub
            )

            # --- emission helpers -------------------------------------
            # score/projection psum tiles cycle through the 2-buffer pool;
            # WAR hazards between consecutive users are tracked per tile.
            slot_counter = [0]

            def next_slot():
                slot_counter[0] += 1
                return ps_s.tile(
                    [128, S], f32, tag="S", name=f"S{slot_counter[0]}"
                )

            def qk_mtile(w, m, pack_eng, ps=None):
                """Q/K projection m-tile into a psum slot, pack to fp8."""
                dstT = qT4 if w == "q" else kT4
                thr = thrq_sb if w == "q" else thrk_sb
                if ps is None:
                    ps = next_slot()
                for half in range(2):
                    sl = slice(half * 512, (half + 1) * 512)
                    for c2 in range(KC // 2):
                        nc.tensor.matmul(
                            ps[:, sl],
                            lhsT=swT[w][
                                :, 2 * c2 : 2 * c2 + 2, m * 128 : (m + 1) * 128
                            ],
                            rhs=shT[:, 2 * c2 : 2 * c2 + 2, sl],
                            start=(c2 == 0),
                            stop=(c2 == KC // 2 - 1),
                            perf_mode=DR,
                        )
                pack_eng.tensor_scalar(
                    dstT[:, m // 2, m % 2, :], ps, thr[:, m : m + 1], 0.5,
                    gt, sub,
                )

            def v_ttile(t, pack_eng=None):
                """V projection token-tile via a Cps half-slot; bf16 for
                t<4 (bf16 ctx chunks), fp8 for t>=4 (DoubleRow ctx pairs)."""
                ps = Cps[:, (t % 2) * 512 : (t % 2 + 1) * 512]
                for c2 in range(KC // 2):
                    nc.tensor.matmul(
                        ps,
                        lhsT=shT[
                            :, 2 * c2 : 2 * c2 + 2, t * 128 : (t + 1) * 128
                        ],
                        rhs=swT["v"][:, 2 * c2 : 2 * c2 + 2, :],
                        start=(c2 == 0),
                        stop=False,
                        perf_mode=DR,
                    )
                nc.tensor.matmul(
                    ps, lhsT=ones16, rhs=bvrow16, start=False, stop=True
                )
                dst = v_bf[:, t, :] if t < 4 else v_q8[:, t - 4, :]
                (pack_eng or nc.gpsimd).tensor_scalar(
                    dst, ps, 0.0, 0.5, gt, sub
                )

            # head-local state for the pipelined tail
            hstate = {}

            def sc_exp(h, c):
                st = hstate[h]
                ps = next_slot()
                kv, qv = st["kv"], st["qv"]
                for sp in range(2):
                    sl = slice(sp * 512, (sp + 1) * 512)
                    nc.tensor.matmul(
                        ps[:, sl],
                        lhsT=kv[:, :, c * 128 : (c + 1) * 128],
                        rhs=qv[:, :, sl],
                        start=True,
                        stop=True,
                        perf_mode=DR,
                        tile_position=(32 * st["b"], 0),
                    )
                if c < 6 or h == NH // G - 1:
                    nc.scalar.activation(
                        st["E"][:, c, :], ps, Exp,
                        bias=mask_sb[:, c : c + 1], scale=exp_scale,
                    )
                else:
                    eeng = nc.gpsimd
                    eeng.add_instruction(
                        mybir.InstActivation(
                            name=nc.get_next_instruction_name(),
                            func=Exp,
                            ins=[
                                eeng.lower_ap(ps),
                                eeng.lower_ap(mask_sb[:, c : c + 1]),
                                mybir.ImmediateValue(
                                    dtype=f32, value=exp_scale
                                ),
                                mybir.ImmediateValue(dtype=f32, value=0.0),
                            ],
                            outs=[eeng.lower_ap(st["E"][:, c, :])],
                        )
                    )

            def fold_sum(h, p, direct=False):
                st = hstate[h]
                if direct:
                    # skip the fold: sum the two raw chunks (shorter latency
                    # for the final head's tail at slightly more PE work)
                    for ci in range(2):
                        for sp in range(2):
                            sl = slice(sp * 512, (sp + 1) * 512)
                            nc.tensor.matmul(
                                Tps[:, sl],
                                lhsT=onesK,
                                rhs=st["E"][:, 2 * p + ci, sl],
                                start=(p == 0 and ci == 0),
                                stop=(p == TC // 2 - 1 and ci == 1),
                            )
                    return
                nc.vector.tensor_tensor(
                    st["Efold"][:, p, :],
                    st["E"][:, 2 * p, :],
                    st["E"][:, 2 * p + 1, :],
                    add,
                )
                for sp in range(2):
                    sl = slice(sp * 512, (sp + 1) * 512)
                    nc.tensor.matmul(
                        Tps[:, sl],
                        lhsT=onesK,
                        rhs=st["Efold"][:, p, sl],
                        start=(p == 0),
                        stop=(p == TC // 2 - 1),
                    )

            def th(h):
                st = hstate[h]
                # halves on DVE and Pool in parallel to halve the latency
                nc.vector.tensor_scalar(
                    st["Th"][:, 0:512], Tps[:, 0:512], th_scale, None, mult
                )
                nc.gpsimd.tensor_scalar(
                    st["Th"][:, 512:1024], Tps[:, 512:1024],
                    th_scale, None, mult,
                )

            def cmp_ctx(h, c, start_c, stop_c):
                st = hstate[h]
                hl, hp = st["hl"], st["hp"]
                if c < 4:
                    nc.vector.tensor_tensor(
                        st["P"][:, c, :], st["E"][:, c, :], st["Th"], gt
                    )
                    for sp in range(2):
                        sl = slice(sp * 512, (sp + 1) * 512)
                        nc.tensor.matmul(
                            Cps[hp : hp + 64, sl],
                            lhsT=v_bf[:, c, hl * 64 : (hl + 1) * 64],
                            rhs=st["P"][:, c, sl],
                            start=(c == start_c),
                            stop=(c == stop_c),
                            tile_position=(0, hp),
                        )
                else:
                    nc.gpsimd.tensor_tensor(
                        st["Pq"][:, c - 4, :], st["E"][:, c, :], st["Th"], gt
                    )
                    if c in (5, 7):  # DoubleRow pair (c-1, c)
                        pr = (c - 5) // 2  # 0 or 1
                        for sp in range(2):
                            sl = slice(sp * 512, (sp + 1) * 512)
                            nc.tensor.matmul(
                                Cps[hp : hp + 64, sl],
                                lhsT=v_q8[
                                    :, 2 * pr : 2 * pr + 2,
                                    hl * 64 : (hl + 1) * 64,
                                ],
                                rhs=st["Pq"][:, 2 * pr : 2 * pr + 2, sl],
                                start=(c == start_c),
                                stop=(c == stop_c),
                                perf_mode=DR,
                                tile_position=(0, hp),
                            )

            def out_m(m):
                for half in range(2):
                    sl = slice(half * 512, (half + 1) * 512)
                    eng = nc.gpsimd if half == 0 else nc.vector
                    eng.tensor_scalar(
                        out_sb[:, m, sl], Cps[:, sl], out_scale, None, mult
                    )
                    nc.sync.dma_start(
                        out=out_d.rearrange("(m p) s -> p m s", p=128)[
                            :, m, sl
                        ],
                        in_=out_sb[:, m, sl],
                    )

            def open_head(h):
                hl = h
                b, hq = hl % 4, hl // 4
                hstate[h] = {
                    "hl": hl,
                    "b": b,
                    "hp": 64 * (hl % 2),
                    "kv": kT4[32 * b : 32 * b + 32, hq, :, :],
                    "qv": qT4[32 * b : 32 * b + 32, hq, :, :],
                    "E": headp.tile([128, TC, S], bf16, tag="E", name=f"E_{h}"),
                    "Efold": headp.tile(
                        [128, TC // 2, S], bf16, tag="Ef", name=f"Ef_{h}"
                    ),
                    "P": headp.tile([128, 4, S], bf16, tag="P", name=f"P_{h}"),
                    "Pq": headp.tile(
                        [128, 4, S], fp8, tag="Pq", name=f"Pq_{h}"
                    ),
                    "Th": headp.tile([128, S], bf16, tag="Th", name=f"Th_{h}"),
                }

            # --- prelude QKV: m-tiles 0,1 of k then q (enough for heads
            # 0-3); k1 borrows the Tps banks so all four tiles pipeline
            # without slot-reuse stalls; packs split across DVE and Pool.
            qk_mtile("k", 0, nc.vector)
            qk_mtile("k", 1, nc.gpsimd, ps=Tps)
            qk_mtile("q", 0, nc.gpsimd)
            qk_mtile("q", 1, nc.vector)

            # extras: v-projection fills head-0's window; the B-half w
            # packs and late QKV m-tiles (needed first by head 4) spread
            # thinly over the head-1/2 windows so their packs never crowd
            # the per-chunk DVE/Pool budget.
            def swB_pack(w):
                nc.vector.tensor_scalar(
                    swT[w][:, 0:4, AHG:HG], wstage[w][:, 0:4, AHG:HG],
                    0.0, 0.5, gt, sub,
                )
                nc.gpsimd.tensor_scalar(
                    swT[w][:, 4:8, AHG:HG], wstage[w][:, 4:8, AHG:HG],
                    0.0, 0.5, gt, sub,
                )

            extra_jobs = {}
            for t in range(4):
                extra_jobs[(0, (1, 3, 5, 6)[t])] = lambda t=t: v_ttile(t)
            for t in range(4, TC):
                extra_jobs[(1, t - 4)] = lambda t=t: v_ttile(
                    t, nc.vector if t < 6 else nc.gpsimd
                )
            extra_jobs[(2, 0)] = lambda: swB_pack("k")
            extra_jobs[(2, 2)] = lambda: swB_pack("q")
            extra_jobs[(2, 4)] = lambda: qk_mtile("q", 2, nc.vector)
            extra_jobs[(2, 6)] = lambda: qk_mtile("k", 2, nc.gpsimd)
            extra_jobs[(3, 0)] = lambda: qk_mtile("q", 3, nc.vector)
            extra_jobs[(3, 2)] = lambda: qk_mtile("k", 3, nc.gpsimd)

            # --- pipelined head loop ----------------------------------
            # Position schedule per window (head h), lag-1 for head h-1:
            #   c0,c1: scores->slots, pool exps of h-1 finishing up
            #   c1:    sum p3(h-1) + th(h-1)
            #   c2:    scores->Tps; pool compare c'=4 of h-1
            #   c3-c6: compares of h-1 (pool c'=5..7, DVE c'=0..3) + ctx
            #   c4,c5,c7: sums p0,p1,p2 of h
            #   c7:    output pack/DMA for a completed m-group
            for h in range(NH // G):
                open_head(h)
                for c in range(TC):
                    sc_exp(h, c)
                    if h >= 1 and c == 1:
                        fold_sum(h - 1, 3)
                        th(h - 1)
                    if c == 4:
                        fold_sum(h, 0)
                    elif c == 6:
                        fold_sum(h, 1)
                    elif c == 7:
                        fold_sum(h, 2)
                    job = extra_jobs.get((h, c))
                    if job is not None:
                        job()
                    if h >= 1:
                        hp_ = h - 1
                        if c == 2:
                            cmp_ctx(hp_, 4, 5, 3)
                        elif c == 3:
                            cmp_ctx(hp_, 5, 5, 3)
                            cmp_ctx(hp_, 0, 5, 3)
                        elif c == 4:
                            cmp_ctx(hp_, 6, 5, 3)
                            cmp_ctx(hp_, 1, 5, 3)
                        elif c == 5:
                            cmp_ctx(hp_, 7, 5, 3)
                            cmp_ctx(hp_, 2, 5, 3)
                        elif c == 6:
                            cmp_ctx(hp_, 3, 5, 3)
                        elif c == 7 and hp_ % 2 == 1:
                            out_m(hp_ // 2)
                del_h = h - 2
                if del_h in hstate:
                    del hstate[del_h]
            # tail: last head's sum/compare/context chain
            hlast = NH // G - 1
            fold_sum(hlast, 3, direct=True)
            th(hlast)
            for c in range(TC):
                cmp_ctx(hlast, c, 0, 7)
            out_m(MC - 1)
    return _split_multi_waits(nc)


_CACHE = {}


def _get_program(exp_scale, th_scale, out_scale, masked):
    key = (exp_scale, th_scale, out_scale, masked)
    if key not in _CACHE:
        _CACHE[key] = _build_program(exp_scale, th_scale, out_scale, masked)
    return _CACHE[key]


def make_in_maps(
    hidden_states,
    attention_mask,
    Wq,
    bq,
    Wk,
    bk,
    Wv,
    bv,
    a_q,
    a_k,
    a_v,
    clip_query,
    clip_key,
    clip_value,
    clip_attn,
):
    """Host-side marshalling: shard (pre-transposed, sign-preserving bf16
    delivery, q/k column fold-permutation) + fold scalar thresholds."""
    import ml_dtypes

    bf16 = ml_dtypes.bfloat16

    aq = max(float(np.asarray(a_q).reshape(-1)[0]), EPS)
    ak = max(float(np.asarray(a_k).reshape(-1)[0]), EPS)
    av = max(float(np.asarray(a_v).reshape(-1)[0]), EPS)
    cq = max(float(np.asarray(clip_query).reshape(-1)[0]), EPS)
    ck = max(float(np.asarray(clip_key).reshape(-1)[0]), EPS)
    cv = max(float(np.asarray(clip_value).reshape(-1)[0]), EPS)
    ca = max(float(np.asarray(clip_attn).reshape(-1)[0]), EPS)
    sq = float(np.abs(Wq).mean())
    sk = float(np.abs(Wk).mean())
    sv = float(np.abs(Wv).mean())

    # packed signs are +-0.5 -> matmul results are M/4
    thrq_full = (-bq / (4.0 * aq * sq)).astype(np.float32)
    thrk_full = (-bk / (4.0 * ak * sk)).astype(np.float32)
    bvrow_full = (bv / (4.0 * av * sv)).astype(np.float32)

    exp_scale = cq * ck * 0.5
    th_scale = 0.5 * ca
    out_scale = 2.0 * ca * cv

    mask = np.ascontiguousarray(
        np.asarray(attention_mask, dtype=np.float32).reshape(B, S)
    )
    masked = bool(np.any(mask != 0.0))

    hs = np.asarray(hidden_states, dtype=np.float32)
    hT = [np.ascontiguousarray(hs[b].T.astype(bf16)) for b in range(B)]
    WT = {
        "q": np.asarray(Wq, np.float32).T,
        "k": np.asarray(Wk, np.float32).T,
        "v": np.asarray(Wv, np.float32).T,
    }
    in_maps = []
    for core in range(NCORES):
        b, g = divmod(core, G)
        sl = slice(g * HG, (g + 1) * HG)
        wq_sh = WT["q"][:, sl]
        wk_sh = WT["k"][:, sl]
        in_maps.append(
            {
                "hT": hT[b],
                "wqT": np.ascontiguousarray(wq_sh[:, _PERM].astype(bf16)),
                "wkT": np.ascontiguousarray(wk_sh[:, _PERM].astype(bf16)),
                "wvT": np.ascontiguousarray(WT["v"][:, sl].astype(bf16)),
                "thrq": np.ascontiguousarray(thrq_full[sl][_PERM]),
                "thrk": np.ascontiguousarray(thrk_full[sl][_PERM]),
                "bvrow": np.ascontiguousarray(bvrow_full[sl]),
                "mask": mask[b],
            }
        )
    return in_maps, (exp_scale, th_scale, out_scale, masked)


def assemble_output(results):
    """Unshard: per-core ctxT [HG, S] -> [B, S, H] (transpose + concat)."""
    out = np.empty((B, S, H), dtype=np.float32)
    for core, res in enumerate(results):
        b, g = divmod(core, G)
        out[b, :, g * HG : (g + 1) * HG] = res["ctxT"].T
    return out


def kernel(**inputs) -> np.ndarray:
    from concourse.bass_utils import run_bass_kernel_spmd

    in_maps, scales = make_in_maps(**inputs)
    nc = _get_program(*scales)
    res = run_bass_kernel_spmd(nc, in_maps, list(range(NCORES)))
    return assemble_output(res.results)


# revision 18
# speedup vs baseline: 1.0083x; 1.0044x over previous
"""Trainium2 Bass kernel for binarized BERT self-attention (BiT-style).

Reference math:
  q = sign(h)*a_q @ (sign(Wq)*mean|Wq|).T + bq     (binarized linear)
  q2 = sign(q)*clip_q   (same for k, v)
  p  = softmax(q2 k2^T / sqrt(D) + mask)
  pq = clip(round(p/clip_a), 0, 1) * clip_a        (binary attention probs)
  out = pq @ v2

Exact algebraic facts used:
  * sign values pack as +-0.5 (exact in fp8e4m3/bf16); every matmul here is
    exact small-integer arithmetic accumulated in fp32 PSUM.
  * sign(x@W.T + b) thresholds the packed-sign matmul: (M/4) > -b/(4*a*s).
  * pq nonzero iff exp(s_i) > 0.5*clip_a * sum_j exp(s_j); scores bounded so
    no softmax max-pass is needed.  round() half-to-even matches strict '>'.

Sharding (8 cores): core = (batch b, head-group g); each core computes 8
heads x 1024 tokens and returns ctx^T [512 head-cols, 1024 tokens].

Performance structure (CoreSim cost-model driven, software-pipelined):
  * Scores run fp8 DoubleRow (0.5 cyc/row): the D=64 contraction is folded
    as [Ki=32 partitions, Ko=2]; Wq/Wk columns are host-permuted so each
    projection m-tile lands the fold layout directly.
  * exp() split across Scalar (chunks 0-6) and Pool (chunk 7) engines.
  * Softmax-sum matmul runs over pair-folded E (DVE folds) -> half PE work.
  * Prob compares: chunks 0-3 on DVE (bf16 P), chunks 4-7 on Pool writing
    fp8 P, so half the context matmul runs fp8 DoubleRow.
  * Emission is software-pipelined: head h's exp stream overlaps head h-1's
    compare/context tail, V-projection and late QKV tiles fill the first
    two head windows, and all DMAs are batched with the critical h/wq/wk
    bytes front-loaded (bf16 delivery, sign-preserving).
"""

import math

import numpy as np

B, S, H, NH, D = 4, 1024, 1024, 16, 64
NCORES, G = 8, 2
HG = H // G  # 512 output columns per core (8 heads)
EPS = 1e-5
KC = H // 128  # 8 contraction chunks
TC = S // 128  # 8 token chunks
MC = HG // 128  # 4 output chunks per core


def _qk_perm():
    """Column order for Wq/Wk shards: m-tile m=(2*hq+o), partition p=32*b+r
    holds neuron (4*hq+b)*64 + o*32 + r, so the projection psum directly
    yields the [Ki=32, Ko=2] DoubleRow fold for the scores contraction."""
    perm = np.empty(HG, dtype=np.int64)
    for m in range(MC):
        hq, o = m // 2, m % 2
        for p in range(128):
            b, r = p // 32, p % 32
            perm[m * 128 + p] = (4 * hq + b) * 64 + o * 32 + r
    return perm


_PERM = _qk_perm()


def _split_multi_waits(nc):
    """Walrus accepts at most ONE sync-wait per instruction.  Move all but
    one wait onto preceding same-engine NOPs (semantically equivalent since
    engines execute their streams in order)."""
    from concourse import mybir

    eng_api = {
        mybir.EngineType.PE: nc.tensor,
        mybir.EngineType.DVE: nc.vector,
        mybir.EngineType.Activation: nc.scalar,
        mybir.EngineType.Pool: nc.gpsimd,
        mybir.EngineType.SP: nc.sync,
    }

    plan = []
    for f in nc.m.functions:
        for bb in f.blocks:
            for ins in bb.instructions:
                si = ins.sync_info
                if si is None or not si.on_wait or len(si.on_wait) <= 1:
                    continue
                plan.append((f, bb, ins))

    fillers = {}
    for f, bb, ins in plan:
        si = ins.sync_info
        waits = list(si.on_wait)
        nops = []
        for w in waits[:-1]:
            bi = eng_api[ins.engine].nop()
            raw = bi.ins
            raw.sync_info = mybir.SyncInfo(on_wait=[w], on_update=[])
            nops.append(raw)
        ins.sync_info = mybir.SyncInfo(
            on_wait=[waits[-1]], on_update=list(si.on_update or [])
        )
        fillers[ins.name] = nops

    created = {n.name for nops in fillers.values() for n in nops}
    for f in nc.m.functions:
        for bb in f.blocks:
            out = []
            for ins in bb.instructions:
                if ins.name in created:
                    continue
                out.extend(fillers.get(ins.name, ()))
                out.append(ins)
            bb.instructions = out
    return nc


def _build_program(exp_scale: float, th_scale: float, out_scale: float,
                   masked: bool):
    import concourse.bass as bass
    import concourse.tile as tile
    from concourse import mybir

    f32, bf16 = mybir.dt.float32, mybir.dt.bfloat16
    fp8 = mybir.dt.float8e4
    DR = mybir.MatmulPerfMode.DoubleRow
    gt = mybir.AluOpType.is_gt
    sub = mybir.AluOpType.subtract
    mult = mybir.AluOpType.mult
    add = mybir.AluOpType.add
    Exp = mybir.ActivationFunctionType.Exp

    nc = bass.Bass()
    hT_d = nc.dram_tensor("hT", [H, S], bf16, kind="ExternalInput")
    wT_d = {
        w: nc.dram_tensor(f"w{w}T", [H, HG], bf16, kind="ExternalInput")
        for w in "qkv"
    }
    thrq_d = nc.dram_tensor("thrq", [HG], f32, kind="ExternalInput")
    thrk_d = nc.dram_tensor("thrk", [HG], f32, kind="ExternalInput")
    bvrow_d = nc.dram_tensor("bvrow", [HG], f32, kind="ExternalInput")
    mask_d = nc.dram_tensor("mask", [S], f32, kind="ExternalInput")
    out_d = nc.dram_tensor("ctxT", [HG, S], f32, kind="ExternalOutput")

    with tile.TileContext(nc) as tc:
        with (
            tc.tile_pool(name="persist", bufs=1) as persist,
            tc.tile_pool(name="heads", bufs=2) as headp,
            tc.tile_pool(name="ps_s", bufs=2, space="PSUM") as ps_s,
            tc.tile_pool(name="ps_t", bufs=1, space="PSUM") as ps_t,
            tc.tile_pool(name="ps_c", bufs=1, space="PSUM") as ps_c,
        ):
            hstage = persist.tile([128, KC, S], bf16, tag="hstage")
            wstage = {
                w: persist.tile(
                    [128, KC, HG], bf16, tag=f"wstage_{w}", name=f"wstage_{w}"
                )
                for w in "qkv"
            }
            shT = persist.tile([128, KC, S], fp8, tag="shT")
            swT = {
                w: persist.tile(
                    [128, KC, HG], fp8, tag=f"swT_{w}", name=f"swT_{w}"
                )
                for w in "qkv"
            }
            # fold layout: [32*b + r, hq, o, token]; head 4*hq+b, d = o*32+r
            qT4 = persist.tile([128, 2, 2, S], fp8, tag="qT4")
            kT4 = persist.tile([128, 2, 2, S], fp8, tag="kT4")
            v_bf = persist.tile([128, 4, HG], bf16, tag="v_bf")  # t=0..3
            v_q8 = persist.tile([128, 4, HG], fp8, tag="v_q8")  # t=4..7
            thrq_sb = persist.tile([128, MC], f32, tag="thrq")
            thrk_sb = persist.tile([128, MC], f32, tag="thrk")
            bvrow_sb = persist.tile([1, HG], f32, tag="bvrow")
            bvrow16 = persist.tile([1, HG], bf16, tag="bvrow16")
            mask_sb = persist.tile([128, TC], f32, tag="mask")
            ones16 = persist.tile([1, 128], bf16, tag="ones16")
            onesK = persist.tile([128, 128], bf16, tag="onesK")
            out_sb = persist.tile([128, MC, S], f32, tag="out_sb")

            # score/projection psum comes from a 2-buffer pool (2 banks per
            # buffer -> 4 banks total); Tps/Cps are persistent single tiles.
            Tps = ps_t.tile([128, S], f32, tag="Tps")  # 2 banks
            Cps = ps_c.tile([128, S], f32, tag="Cps")  # 2 banks

            # warmup: memsets on DVE, then dummy matmuls keep the PE
            # continuously busy through the load phase so the p-state ramp
            # completes before the first real projection; a dummy exp
            # pre-loads the activation table off the critical path.
            nc.vector.memset(onesK, 1.0)
            nc.vector.memset(ones16, 1.0)
            warm_sb = persist.tile([1, 128], bf16, tag="warm_sb")
            nc.scalar.activation(warm_sb, ones16, Exp, bias=0.0, scale=0.0)
            for _ in range(120):
                nc.tensor.matmul(
                    Tps[:, 0:128], lhsT=onesK, rhs=onesK,
                    start=True, stop=True,
                )
            # tiny control tensors first so their (negligible) transfers
            # clear the DMA device before the bulk loads queue up.
            nc.gpsimd.dma_start(
                out=thrq_sb, in_=thrq_d.rearrange("(m p) -> p m", p=128)
            )
            nc.gpsimd.dma_start(
                out=thrk_sb, in_=thrk_d.rearrange("(m p) -> p m", p=128)
            )
            nc.gpsimd.dma_start(
                out=bvrow_sb, in_=bvrow_d.rearrange("(o n) -> o n", o=1)
            )
            nc.gpsimd.memset(mask_sb, 0.0)
            if masked:
                nc.gpsimd.dma_start(
                    out=mask_sb, in_=mask_d.rearrange("(t p) -> p t", p=128)
                )
            nc.vector.tensor_copy(bvrow16, bvrow_sb)

            # --- input DMAs.  All bulk loads issue on SP in strict
            # priority order (h quarters, then the m0/m1 halves of wq/wk,
            # then the rest) -- a single issuer guarantees DMA-device order
            # so nothing steals slots from the critical path.
            AHG = HG // 2  # 256: columns of m-tiles 0,1
            for qtr in range(4):
                nc.sync.dma_start(
                    out=hstage[:, 2 * qtr : 2 * qtr + 2, :],
                    in_=hT_d[qtr * 256 : (qtr + 1) * 256, :].rearrange(
                        "(c p) s -> p c s", p=128
                    ),
                )
                eng = nc.vector if qtr % 2 == 0 else nc.gpsimd
                eng.tensor_scalar(
                    shT[:, 2 * qtr : 2 * qtr + 2, :],
                    hstage[:, 2 * qtr : 2 * qtr + 2, :],
                    0.0,
                    0.5,
                    gt,
                    sub,
                )
            for w in ("k", "q"):
                nc.sync.dma_start(
                    out=wstage[w][:, :, 0:AHG],
                    in_=wT_d[w][:, 0:AHG].rearrange("(c p) o -> p c o", p=128),
                )
                nc.vector.tensor_scalar(
                    swT[w][:, :, 0:AHG], wstage[w][:, :, 0:AHG],
                    0.0, 0.5, gt, sub,
                )
            for w in ("k", "q"):
                nc.sync.dma_start(
                    out=wstage[w][:, :, AHG:HG],
                    in_=wT_d[w][:, AHG:HG].rearrange(
                        "(c p) o -> p c o", p=128
                    ),
                )
            nc.sync.dma_start(
                out=wstage["v"],
                in_=wT_d["v"].rearrange("(c p) o -> p c o", p=128),
            )
            nc.vector.tensor_scalar(
                swT["v"], wstage["v"], 0.0, 0.5, gt, sub
            )

            # --- emission helpers -------------------------------------
            # score/projection psum tiles cycle through the 2-buffer pool;
            # WAR hazards between consecutive users are tracked per tile.
            slot_counter = [0]

            def next_slot():
                slot_counter[0] += 1
                return ps_s.tile(
                    [128, S], f32, tag="S", name=f"S{slot_counter[0]}"
                )

            def qk_mtile(w, m, pack_eng, ps=None):
                """Q/K projection m-tile into a psum slot, pack to fp8."""
                dstT = qT4 if w == "q" else kT4
                thr = thrq_sb if w == "q" else thrk_sb
                if ps is None:
                    ps = next_slot()
                for half in range(2):
                    sl = slice(half * 512, (half + 1) * 512)
                    for c2 in range(KC // 2):
                        nc.tensor.matmul(
                            ps[:, sl],
                            lhsT=swT[w][
                                :, 2 * c2 : 2 * c2 + 2, m * 128 : (m + 1) * 128
                            ],
                            rhs=shT[:, 2 * c2 : 2 * c2 + 2, sl],
                            start=(c2 == 0),
                            stop=(c2 == KC // 2 - 1),
                            perf_mode=DR,
                        )
                pack_eng.tensor_scalar(
                    dstT[:, m // 2, m % 2, :], ps, thr[:, m : m + 1], 0.5,
                    gt, sub,
                )

            def v_ttile(t, pack_eng=None):
                """V projection token-tile via a Cps half-slot; bf16 for
                t<4 (bf16 ctx chunks), fp8 for t>=4 (DoubleRow ctx pairs)."""
                ps = Cps[:, (t % 2) * 512 : (t % 2 + 1) * 512]
                for c2 in range(KC // 2):
                    nc.tensor.matmul(
                        ps,
                        lhsT=shT[
                            :, 2 * c2 : 2 * c2 + 2, t * 128 : (t + 1) * 128
                        ],
                        rhs=swT["v"][:, 2 * c2 : 2 * c2 + 2, :],
                        start=(c2 == 0),
                        stop=False,
                        perf_mode=DR,
                    )
                nc.tensor.matmul(
                    ps, lhsT=ones16, rhs=bvrow16, start=False, stop=True
                )
                dst = v_bf[:, t, :] if t < 4 else v_q8[:, t - 4, :]
                (pack_eng or nc.gpsimd).tensor_scalar(
                    dst, ps, 0.0, 0.5, gt, sub
                )

            # head-local state for the pipelined tail
            hstate = {}

            def sc_exp(h, c):
                st = hstate[h]
                ps = next_slot()
                kv, qv = st["kv"], st["qv"]
                for sp in range(2):
                    sl = slice(sp * 512, (sp + 1) * 512)
                    nc.tensor.matmul(
                        ps[:, sl],
                        lhsT=kv[:, :, c * 128 : (c + 1) * 128],
                        rhs=qv[:, :, sl],
                        start=True,
                        stop=True,
                        perf_mode=DR,
                        tile_position=(32 * st["b"], 0),
                    )
                if c < 6 or h == NH // G - 1:
                    nc.scalar.activation(
                        st["E"][:, c, :], ps, Exp,
                        bias=mask_sb[:, c : c + 1], scale=exp_scale,
                    )
                else:
                    eeng = nc.gpsimd
                    eeng.add_instruction(
                        mybir.InstActivation(
                            name=nc.get_next_instruction_name(),
                            func=Exp,
                            ins=[
                                eeng.lower_ap(ps),
                                eeng.lower_ap(mask_sb[:, c : c + 1]),
                                mybir.ImmediateValue(
                                    dtype=f32, value=exp_scale
                                ),
                                mybir.ImmediateValue(dtype=f32, value=0.0),
                            ],
                            outs=[eeng.lower_ap(st["E"][:, c, :])],
                        )
                    )

            def fold_sum(h, p, direct=False):
                st = hstate[h]
                if direct:
                    # skip the fold: sum the two raw chunks (shorter latency
                    # for the final head's tail at slightly more PE work)
                    for ci in range(2):
                        for sp in range(2):
                            sl = slice(sp * 512, (sp + 1) * 512)
                            nc.tensor.matmul(
                                Tps[:, sl],
                                lhsT=onesK,
                                rhs=st["E"][:, 2 * p + ci, sl],
                                start=(p == 0 and ci == 0),
                                stop=(p == TC // 2 - 1 and ci == 1),
                            )
                    return
                nc.vector.tensor_tensor(
                    st["Efold"][:, p, :],
                    st["E"][:, 2 * p, :],
                    st["E"][:, 2 * p + 1, :],
                    add,
                )
                for sp in range(2):
                    sl = slice(sp * 512, (sp + 1) * 512)
                    nc.tensor.matmul(
                        Tps[:, sl],
                        lhsT=onesK,
                        rhs=st["Efold"][:, p, sl],
                        start=(p == 0),
                        stop=(p == TC // 2 - 1),
                    )

            def th(h):
                st = hstate[h]
                # halves on DVE and Pool in parallel to halve the latency
                nc.vector.tensor_scalar(
                    st["Th"][:, 0:512], Tps[:, 0:512], th_scale, None, mult
                )
                nc.gpsimd.tensor_scalar(
                    st["Th"][:, 512:1024], Tps[:, 512:1024],
                    th_scale, None, mult,
                )

            def cmp_ctx(h, c, start_c, stop_c):
                st = hstate[h]
                hl, hp = st["hl"], st["hp"]
                if c < 4:
                    nc.vector.tensor_tensor(
                        st["P"][:, c, :], st["E"][:, c, :], st["Th"], gt
                    )
                    for sp in range(2):
                        sl = slice(sp * 512, (sp + 1) * 512)
                        nc.tensor.matmul(
                            Cps[hp : hp + 64, sl],
                            lhsT=v_bf[:, c, hl * 64 : (hl + 1) * 64],
                            rhs=st["P"][:, c, sl],
                            start=(c == start_c),
                            stop=(c == stop_c),
                            tile_position=(0, hp),
                        )
                else:
                    nc.gpsimd.tensor_tensor(
                        st["Pq"][:, c - 4, :], st["E"][:, c, :], st["Th"], gt
                    )
                    if c in (5, 7):  # DoubleRow pair (c-1, c)
                        pr = (c - 5) // 2  # 0 or 1
                        for sp in range(2):
                            sl = slice(sp * 512, (sp + 1) * 512)
                            nc.tensor.matmul(
                                Cps[hp : hp + 64, sl],
                                lhsT=v_q8[
                                    :, 2 * pr : 2 * pr + 2,
                                    hl * 64 : (hl + 1) * 64,
                                ],
                                rhs=st["Pq"][:, 2 * pr : 2 * pr + 2, sl],
                                start=(c == start_c),
                                stop=(c == stop_c),
                                perf_mode=DR,
                                tile_position=(0, hp),
                            )

            def out_m(m):
                for half in range(2):
                    sl = slice(half * 512, (half + 1) * 512)
                    eng = nc.gpsimd if half == 0 else nc.vector
                    eng.tensor_scalar(
                        out_sb[:, m, sl], Cps[:, sl], out_scale, None, mult
                    )
                    nc.sync.dma_start(
                        out=out_d.rearrange("(m p) s -> p m s", p=128)[
                            :, m, sl
                        ],
                        in_=out_sb[:, m, sl],
                    )

            def open_head(h):
                hl = h
                b, hq = hl % 4, hl // 4
                hstate[h] = {
                    "hl": hl,
                    "b": b,
                    "hp": 64 * (hl % 2),
                    "kv": kT4[32 * b : 32 * b + 32, hq, :, :],
                    "qv": qT4[32 * b : 32 * b + 32, hq, :, :],
                    "E": headp.tile([128, TC, S], bf16, tag="E", name=f"E_{h}"),
                    "Efold": headp.tile(
                        [128, TC // 2, S], bf16, tag="Ef", name=f"Ef_{h}"
                    ),
                    "P": headp.tile([128, 4, S], bf16, tag="P", name=f"P_{h}"),
                    "Pq": headp.tile(
                        [128, 4, S], fp8, tag="Pq", name=f"Pq_{h}"
                    ),
                    "Th": headp.tile([128, S], bf16, tag="Th", name=f"Th_{h}"),
                }

            # --- prelude QKV: m-tiles 0,1 of k then q (enough for heads
            # 0-3); k1 borrows the Tps banks so all four tiles pipeline
            # without slot-reuse stalls; packs split across DVE and Pool.
            qk_mtile("k", 0, nc.vector)
            qk_mtile("k", 1, nc.gpsimd, ps=Tps)
            qk_mtile("q", 0, nc.gpsimd)
            qk_mtile("q", 1, nc.vector)

            # extras: v-projection fills head-0's window; the B-half w
            # packs and late QKV m-tiles (needed first by head 4) spread
            # thinly over the head-1/2 windows so their packs never crowd
            # the per-chunk DVE/Pool budget.
            def swB_pack(w):
                nc.vector.tensor_scalar(
                    swT[w][:, 0:4, AHG:HG], wstage[w][:, 0:4, AHG:HG],
                    0.0, 0.5, gt, sub,
                )
                nc.gpsimd.tensor_scalar(
                    swT[w][:, 4:8, AHG:HG], wstage[w][:, 4:8, AHG:HG],
                    0.0, 0.5, gt, sub,
                )

            extra_jobs = {}
            for t in range(4):
                extra_jobs[(0, (1, 3, 5, 6)[t])] = lambda t=t: v_ttile(t)
            for t in range(4, TC):
                extra_jobs[(1, t - 4)] = lambda t=t: v_ttile(
                    t, nc.vector if t < 6 else nc.gpsimd
                )
            extra_jobs[(2, 0)] = lambda: swB_pack("k")
            extra_jobs[(2, 2)] = lambda: swB_pack("q")
            extra_jobs[(2, 4)] = lambda: qk_mtile("q", 2, nc.vector)
            extra_jobs[(2, 6)] = lambda: qk_mtile("k", 2, nc.gpsimd)
            extra_jobs[(3, 0)] = lambda: qk_mtile("q", 3, nc.vector)
            extra_jobs[(3, 2)] = lambda: qk_mtile("k", 3, nc.gpsimd)

            # --- pipelined head loop ----------------------------------
            # Position schedule per window (head h), lag-1 for head h-1:
            #   c0,c1: scores->slots, pool exps of h-1 finishing up
            #   c1:    sum p3(h-1) + th(h-1)
            #   c2:    scores->Tps; pool compare c'=4 of h-1
            #   c3-c6: compares of h-1 (pool c'=5..7, DVE c'=0..3) + ctx
            #   c4,c5,c7: sums p0,p1,p2 of h
            #   c7:    output pack/DMA for a completed m-group
            for h in range(NH // G):
                open_head(h)
                for c in range(TC):
                    sc_exp(h, c)
                    if h >= 1 and c == 0:
                        fold_sum(h - 1, 2)
                    if h >= 1 and c == 1:
                        fold_sum(h - 1, 3)
                        th(h - 1)
                    if c == 4:
                        fold_sum(h, 0)
                    elif c == 6:
                        fold_sum(h, 1)
                    job = extra_jobs.get((h, c))
                    if job is not None:
                        job()
                    if h >= 1:
                        hp_ = h - 1
                        if c == 2:
                            cmp_ctx(hp_, 4, 5, 3)
                        elif c == 3:
                            cmp_ctx(hp_, 5, 5, 3)
                            cmp_ctx(hp_, 0, 5, 3)
                        elif c == 4:
                            cmp_ctx(hp_, 6, 5, 3)
                            cmp_ctx(hp_, 1, 5, 3)
                        elif c == 5:
                            cmp_ctx(hp_, 7, 5, 3)
                            cmp_ctx(hp_, 2, 5, 3)
                        elif c == 6:
                            cmp_ctx(hp_, 3, 5, 3)
                        elif c == 7 and hp_ % 2 == 1:
                            out_m(hp_ // 2)
                del_h = h - 2
                if del_h in hstate:
                    del hstate[del_h]
            # tail: last head's sum/compare/context chain
            hlast = NH // G - 1
            fold_sum(hlast, 2)
            fold_sum(hlast, 3, direct=True)
            th(hlast)
            for c in range(TC):
                cmp_ctx(hlast, c, 0, 7)
            out_m(MC - 1)
    return _split_multi_waits(nc)


_CACHE = {}


def _get_program(exp_scale, th_scale, out_scale, masked):
    key = (exp_scale, th_scale, out_scale, masked)
    if key not in _CACHE:
        _CACHE[key] = _build_program(exp_scale, th_scale, out_scale, masked)
    return _CACHE[key]


def make_in_maps(
    hidden_states,
    attention_mask,
    Wq,
    bq,
    Wk,
    bk,
    Wv,
    bv,
    a_q,
    a_k,
    a_v,
    clip_query,
    clip_key,
    clip_value,
    clip_attn,
):
    """Host-side marshalling: shard (pre-transposed, sign-preserving bf16
    delivery, q/k column fold-permutation) + fold scalar thresholds."""
    import ml_dtypes

    bf16 = ml_dtypes.bfloat16

    aq = max(float(np.asarray(a_q).reshape(-1)[0]), EPS)
    ak = max(float(np.asarray(a_k).reshape(-1)[0]), EPS)
    av = max(float(np.asarray(a_v).reshape(-1)[0]), EPS)
    cq = max(float(np.asarray(clip_query).reshape(-1)[0]), EPS)
    ck = max(float(np.asarray(clip_key).reshape(-1)[0]), EPS)
    cv = max(float(np.asarray(clip_value).reshape(-1)[0]), EPS)
    ca = max(float(np.asarray(clip_attn).reshape(-1)[0]), EPS)
    sq = float(np.abs(Wq).mean())
    sk = float(np.abs(Wk).mean())
    sv = float(np.abs(Wv).mean())

    # packed signs are +-0.5 -> matmul results are M/4
    thrq_full = (-bq / (4.0 * aq * sq)).astype(np.float32)
    thrk_full = (-bk / (4.0 * ak * sk)).astype(np.float32)
    bvrow_full = (bv / (4.0 * av * sv)).astype(np.float32)

    exp_scale = cq * ck * 0.5
    th_scale = 0.5 * ca
    out_scale = 2.0 * ca * cv

    mask = np.ascontiguousarray(
        np.asarray(attention_mask, dtype=np.float32).reshape(B, S)
    )
    masked = bool(np.any(mask != 0.0))

    hs = np.asarray(hidden_states, dtype=np.float32)
    hT = [np.ascontiguousarray(hs[b].T.astype(bf16)) for b in range(B)]
    WT = {
        "q": np.asarray(Wq, np.float32).T,
        "k": np.asarray(Wk, np.float32).T,
        "v": np.asarray(Wv, np.float32).T,
    }
    in_maps = []
    for core in range(NCORES):
        b, g = divmod(core, G)
        sl = slice(g * HG, (g + 1) * HG)
        wq_sh = WT["q"][:, sl]
        wk_sh = WT["k"][:, sl]
        in_maps.append(
            {
                "hT": hT[b],
                "wqT": np.ascontiguousarray(wq_sh[:, _PERM].astype(bf16)),
                "wkT": np.ascontiguousarray(wk_sh[:, _PERM].astype(bf16)),
                "wvT": np.ascontiguousarray(WT["v"][:, sl].astype(bf16)),
                "thrq": np.ascontiguousarray(thrq_full[sl][_PERM]),
                "thrk": np.ascontiguousarray(thrk_full[sl][_PERM]),
                "bvrow": np.ascontiguousarray(bvrow_full[sl]),
                "mask": mask[b],
            }
        )
    return in_maps, (exp_scale, th_scale, out_scale, masked)


def assemble_output(results):
    """Unshard: per-core ctxT [HG, S] -> [B, S, H] (transpose + concat)."""
    out = np.empty((B, S, H), dtype=np.float32)
    for core, res in enumerate(results):
        b, g = divmod(core, G)
        out[b, :, g * HG : (g + 1) * HG] = res["ctxT"].T
    return out


def kernel(**inputs) -> np.ndarray:
    from concourse.bass_utils import run_bass_kernel_spmd

    in_maps, scales = make_in_maps(**inputs)
    nc = _get_program(*scales)
    res = run_bass_kernel_spmd(nc, in_maps, list(range(NCORES)))
    return assemble_output(res.results)
